# revision 33
# baseline (speedup 1.0000x reference)
"""Trainium2 Bass kernel for nn_Graph_Enhance_model (GNN message passing).

Self-contained: hardcodes shapes B=4,F=32,H=8,O=16,D=2048, 8 cores.
Data-parallel over the 128 (b,f) frames: 16 frames per core.

v2: fp8e4m3 DoubleRow for the edge waves + human GRU (validated: final
rel_err ~2.4e-3 vs 2e-2 budget), algebraic rank-trick for propagation
step 2 (edges after step 1 are w * [msg_e | bcast(msg_n)]; the msg_n
half is rank-16 per frame, so step 2 contracts K=1024 + a rank-16
correction), S-node GRU kept bf16, its input-independent half (whh @
[sc4|sf]) interleaved into Phase A so its weight DMA hides under the
compute-bound wave phase.

Scale conventions (power-of-2, folded into host tensors / activation
scale):
  wcat = 64*[We|Wl1]^T fp8      wave psum (step0) = 64 * true
  e0t  = edges fp8 (x1)         wave psum (step1) = 256 * true
  um1t = 4 * (w1 x msg_e1) fp8  c_sb = 64 * (msg_n @ W_R^T) bf16
  diag64 = 4 * w1 bf16 (mask64 host = 64, times wb0 = w/16)
  ghi/ghh = 64*w^T fp8, msum8 = 32*msum, ht8 = 32*h^T -> psum = 2048x
"""

import os
import sys

for _p in ("/opt/trn_rl_repo", "/opt/pypackages"):
    if _p not in sys.path and os.path.isdir(_p):
        sys.path.append(_p)

import numpy as np
import ml_dtypes

import concourse.bass as bass
import concourse.bacc as bacc
import concourse.tile as tile
import concourse.mybir as mybir
from concourse import bass_utils
from concourse.masks import make_identity

BF16 = mybir.dt.bfloat16
FP8 = mybir.dt.float8e4
F32 = mybir.dt.float32
AF = mybir.ActivationFunctionType
ALU = mybir.AluOpType
AX = mybir.AxisListType
DR = mybir.MatmulPerfMode.DoubleRow

NB = ml_dtypes.bfloat16
N8 = ml_dtypes.float8_e4m3fn
NE3 = ml_dtypes.float8_e3m4
FP8E3 = mybir.dt.float8e3

B, F, H, O, D = 4, 32, 8, 16, 2048
NFRAMES = B * F          # 128
NCORES = 8
FPC = NFRAMES // NCORES  # 16 frames per core
ROWS = H * O             # 128 rows per frame
KC = D // 128            # 16 K-chunks
NQ = FPC // 4            # 4 quads of 4 frames

_CACHE = {}


def q8(x, scale):
    return np.clip(np.asarray(x, np.float32) * scale, -240, 240).astype(N8)


def qe3(x, scale):
    return np.clip(np.asarray(x, np.float32) * scale, -15.5, 15.5).astype(NE3)


def _build_nc():
    nc = bacc.Bacc("TRN2", target_bir_lowering=False, debug=False, num_devices=NCORES)

    def din(name, shape, dt):
        return nc.dram_tensor(name, shape, dt, kind="ExternalInput")

    e0t = din("e0t", [NQ, D, 512], FP8)
    ot = din("ot", [D, FPC * O], FP8)
    wnt = din("wnt", [D, D // 2], FP8)
    wnb64 = din("wnb64", [1, D // 2], BF16)
    wcat = din("wcat", [D, D], FP8)
    bl1td = din("bl1t", [128, 8], F32)
    bet64d = din("bet64", [128, 8], F32)
    bet256d = din("bet256", [128, 8], F32)
    wl2 = din("wl2", [128, 8], BF16)
    mask64d = din("mask64", [64, 512], BF16)
    ht8d = din("ht8", [D, FPC * H], FP8)
    h_rmd = din("h_rm", [FPC * H, D], BF16)
    pmatd = din("pmat", [FPC * H, FPC], BF16)
    ghi = din("ghi", [D, 3 * D], FP8)
    ghib = din("ghib", [1, 3 * D], BF16)
    ghh = din("ghh", [D, 3 * D], FP8)
    ghhb = din("ghhb", [1, 3 * D], BF16)
    scsf = din("scsf", [D, 3 * FPC], BF16)
    sc4rmd = din("sc4rm", [FPC, D], F32)
    sfrmd = din("sfrm", [FPC, D], F32)
    gsi = din("gsi", [D, 3 * D], FP8E3)
    gsib = din("gsib", [1, 3 * D], BF16)
    gsh = din("gsh", [D, 3 * D], BF16)
    gshb = din("gshb", [1, 3 * D], BF16)
    outp = nc.dram_tensor("outp", [FPC, D], F32, kind="ExternalOutput")

    from contextlib import ExitStack

    with tile.TileContext(nc) as tc, ExitStack() as ctx:
        glob = ctx.enter_context(tc.tile_pool(name="glob", bufs=1))

        ones_b = glob.tile([1, 512], BF16)
        nc.vector.memset(ones_b, 1.0)
        ident16 = glob.tile([16, 16], BF16)
        make_identity(nc, ident16)

        wl2_sb = glob.tile([128, 8], BF16)
        bl1t_sb = glob.tile([128, 8], F32)
        bet64_sb = glob.tile([128, 8], F32)
        bet256_sb = glob.tile([128, 8], F32)
        mask64_sb = glob.tile([64, 512], BF16)

        msum8 = glob.tile([128, KC, FPC * H], FP8)      # 32 * msum
        ht8_sb = glob.tile([128, KC, FPC * H], FP8)     # 32 * h^T
        scsf_b = glob.tile([128, KC, 3 * FPC], BF16)    # [S_C4^T | 0 | S_f^T]
        gh1_sb = glob.tile([16, 12, 512], BF16)         # whh@sc4 + bhh
        gh2_sb = glob.tile([16, 12, 512], BF16)         # whh@sf + bhh
        ah_sb = glob.tile([128, KC, FPC], FP8E3)        # 2 * All_human^T

        with (
            tc.tile_pool(name="pwcat", bufs=1) as pwcat,
            tc.tile_pool(name="pa", bufs=2) as pa,
            tc.tile_pool(name="pa1", bufs=1) as pa1,
            tc.tile_pool(name="pc0w", bufs=2) as pc0w,
        ):
            wcat_sb = pwcat.tile([128, KC, D], FP8)
            msgn_sb = pa1.tile([128, 8, FPC * O], BF16)   # true msg_n^T (+bn)
            c_sb = pa1.tile([64, NQ, D], BF16)            # 64 * msg_n @ W_R^T
            msum_f = pa1.tile([128, KC, FPC * H], F32)    # true M_sum2^T

            # ---------------- Phase 0: msg_n^T = Wn @ O^T + bn (fp8 DR) ----------------
            with (
                tc.tile_pool(name="p0", bufs=1) as p0,
                tc.tile_pool(name="p0ps", bufs=4, space="PSUM") as p0ps,
            ):
                # DMA issue order = critical-path order: phase-0 inputs, then
                # first edge quad, then wcat (chunked so waves start early),
                # then everything else.
                ot_sb = p0.tile([128, KC, FPC * O], FP8)
                nc.sync.dma_start(out=ot_sb, in_=ot.ap().rearrange("(kc p) n -> p kc n", p=128))
                wn_sb = p0.tile([128, KC, D // 2], FP8)
                nc.sync.dma_start(out=wn_sb, in_=wnt.ap().rearrange("(kc p) m -> p kc m", p=128))
                wnb_sb = p0.tile([1, D // 2], BF16)
                nc.sync.dma_start(out=wnb_sb, in_=wnb64.ap())
                msgn8 = p0.tile([128, 8, FPC * O], FP8)   # 16 * msg_n
                xqs = {}
                for q in range(2):
                    xqs[q] = pa.tile([128, KC, 512], FP8, tag="xq", name="xq")
                    nc.sync.dma_start(out=xqs[q],
                                      in_=e0t.ap()[q].rearrange("(kc p) n -> p kc n", p=128))
                for k2 in range(8):
                    nc.sync.dma_start(
                        out=wcat_sb[:, 2 * k2:2 * k2 + 2, :],
                        in_=wcat.ap()[256 * k2:256 * (k2 + 1), :]
                        .rearrange("(kc p) m -> p kc m", p=128))
                nc.sync.dma_start(out=wl2_sb, in_=wl2.ap())
                nc.sync.dma_start(out=bl1t_sb, in_=bl1td.ap())
                nc.sync.dma_start(out=bet64_sb, in_=bet64d.ap())
                nc.sync.dma_start(out=bet256_sb, in_=bet256d.ap())
                nc.sync.dma_start(out=mask64_sb, in_=mask64d.ap())
                nc.sync.dma_start(out=ht8_sb, in_=ht8d.ap().rearrange("(kc p) n -> p kc n", p=128))
                nc.sync.dma_start(out=scsf_b, in_=scsf.ap().rearrange("(kc p) n -> p kc n", p=128))
                for mt in range(8):
                    pm = p0ps.tile([128, FPC * O], F32, tag="pm")
                    for k2 in range(8):
                        nc.tensor.matmul(pm, lhsT=wn_sb[:, 2 * k2:2 * k2 + 2, mt * 128:(mt + 1) * 128],
                                         rhs=ot_sb[:, 2 * k2:2 * k2 + 2, :],
                                         start=(k2 == 0), stop=False, perf_mode=DR)
                    nc.tensor.matmul(pm, lhsT=wnb_sb[0:1, mt * 128:(mt + 1) * 128],
                                     rhs=ones_b[0:1, 0:FPC * O], start=False, stop=True)
                    nc.scalar.activation(msgn_sb[:, mt, :], pm, AF.Copy, scale=1.0 / 64)
                    nc.scalar.activation(msgn8[:, mt, :], pm, AF.Copy, scale=1.0 / 4)

                # c = msg_n @ [We_R | Wl1_R]^T  (per-q layout, 64*c in bf16)
                for q in range(NQ):
                    for mtile in range(4):
                        pcp = p0ps.tile([64, 512], F32, tag="pc")
                        for k2 in range(4):
                            nc.tensor.matmul(
                                pcp,
                                lhsT=msgn8[:, 2 * k2:2 * k2 + 2, q * 64:(q + 1) * 64],
                                rhs=wcat_sb[:, 8 + 2 * k2:8 + 2 * k2 + 2,
                                            mtile * 512:(mtile + 1) * 512],
                                start=(k2 == 0), stop=(k2 == 3), perf_mode=DR)
                        nc.scalar.activation(c_sb[:, q, mtile * 512:(mtile + 1) * 512],
                                             pcp, AF.Copy, scale=1.0 / 16)

            # ---------------- Phase A: 2 propagation steps + interleaved C0 ----------------
            # C0 = S-GRU whh @ [sc4 | sf] (input-independent), spread across
            # slots; back-loaded so its PE work covers the q=3 DVE tail.
            c0_sched = [[], [0], [1], [2], [3, 4], [5, 6], [7, 8], [9, 10, 11]]

            def emit_c0(j):
                wsh = pc0w.tile([128, KC, 512], BF16, tag="wsh")
                nc.sync.dma_start(out=wsh, in_=gsh.ap()[:, j * 512:(j + 1) * 512]
                                  .rearrange("(kc p) m -> p kc m", p=128))
                bsh = pc0w.tile([1, 512], BF16, tag="bsh")
                nc.sync.dma_start(out=bsh, in_=gshb.ap()[:, j * 512:(j + 1) * 512])
                PH = pc0ps.tile([48, 512], F32, tag="PH")
                for kc in range(KC):
                    nc.tensor.matmul(PH, lhsT=scsf_b[:, kc, :], rhs=wsh[:, kc, :],
                                     start=(kc == 0), stop=False)
                nc.tensor.matmul(PH, lhsT=ones_b[0:1, 0:48], rhs=bsh[0:1, :],
                                 start=False, stop=True)
                nc.scalar.copy(gh1_sb[:, j, :], PH[0:16, :])
                nc.scalar.copy(gh2_sb[:, j, :], PH[32:48, :])

            with tc.tile_pool(name="paps", bufs=4, space="PSUM") as paps, \
                 tc.tile_pool(name="papss", bufs=1, space="PSUM") as papss, \
                 tc.tile_pool(name="pc0ps", bufs=1, space="PSUM") as pc0ps:
                for q in range(NQ):
                    if q in xqs:
                        xq = xqs[q]
                    else:
                        xq = pa.tile([128, KC, 512], FP8, tag="xq")
                        nc.sync.dma_start(out=xq, in_=e0t.ap()[q].rearrange("(kc p) n -> p kc n", p=128))
                    um1t = pa1.tile([128, 8, 512], FP8, tag="um1t")
                    diag64 = pa1.tile([64, 512], BF16, tag="diag")
                    for step in range(2):
                        psc = 1.0 / 64 if step == 0 else 1.0 / 256

                        def wave_mms(pt, mcol):
                            """full K-contraction for output cols mcol*128
                            into psum pt (start..stop)."""
                            if step == 0:
                                for k2 in range(8):
                                    nc.tensor.matmul(
                                        pt, lhsT=wcat_sb[:, 2 * k2:2 * k2 + 2,
                                                         mcol * 128:(mcol + 1) * 128],
                                        rhs=xq[:, 2 * k2:2 * k2 + 2, :],
                                        start=(k2 == 0), stop=(k2 == 7), perf_mode=DR)
                            else:
                                for k2 in range(4):
                                    nc.tensor.matmul(
                                        pt, lhsT=wcat_sb[:, 2 * k2:2 * k2 + 2,
                                                         mcol * 128:(mcol + 1) * 128],
                                        rhs=um1t[:, 2 * k2:2 * k2 + 2, :],
                                        start=(k2 == 0), stop=False, perf_mode=DR)
                                nc.tensor.matmul(
                                    pt, lhsT=c_sb[:, q, mcol * 128:(mcol + 1) * 128],
                                    rhs=diag64, start=False, stop=True)

                        # --- a-wave: relu(X @ Wl1^T + bl1), transposed ---
                        relu_sb = pa1.tile([128, 8, 512], BF16, tag="relu")
                        for mt in range(8, 16):
                            pw_a = paps.tile([128, 512], F32, tag="wave")
                            wave_mms(pw_a, mt)
                            nc.scalar.activation(relu_sb[:, mt - 8, :], pw_a, AF.Relu,
                                                 bias=bl1t_sb[:, mt - 8:mt - 7], scale=psc)
                        # --- logits + softmax over o (groups of 16) ---
                        pl = papss.tile([1, 512], F32, tag="pl")
                        for kc2 in range(8):
                            nc.tensor.matmul(pl, lhsT=wl2_sb[:, kc2:kc2 + 1],
                                             rhs=relu_sb[:, kc2, :], start=(kc2 == 0), stop=(kc2 == 7))
                        pl3 = pl.rearrange("o (g i) -> o g i", i=16)
                        mx = pa1.tile([1, 32], F32, tag="mx")
                        nc.vector.reduce_max(mx, pl3, axis=AX.X)
                        sub = pa1.tile([1, 512], F32, tag="sub")
                        nc.vector.tensor_tensor(sub.rearrange("o (g i) -> o g i", i=16), pl3,
                                                mx.broadcast_to((1, 32, 16)), op=ALU.subtract)
                        nc.scalar.activation(sub, sub, AF.Exp)
                        ex3 = sub.rearrange("o (g i) -> o g i", i=16)
                        sm = pa1.tile([1, 32], F32, tag="sm")
                        nc.vector.reduce_sum(sm, ex3, axis=AX.X)
                        rs = pa1.tile([1, 32], F32, tag="rs")
                        nc.vector.reciprocal(rs, sm)
                        w_sb = pa1.tile([1, 512], BF16, tag="w")
                        nc.vector.tensor_tensor(w_sb.rearrange("o (g i) -> o g i", i=16), ex3,
                                                rs.broadcast_to((1, 32, 16)), op=ALU.mult)
                        # --- msg_e wave; w-broadcast emitted after 2 groups ---
                        e_ps = []
                        wb0 = pa1.tile([128, 512], F32, tag="wb0")
                        wb1 = pa1.tile([128, 512], F32, tag="wb1")

                        def combine(cmt, pe):
                            if step == 0:
                                nc.vector.scalar_tensor_tensor(
                                    out=um1t[:, cmt, :], in0=pe,
                                    scalar=bet64_sb[:, cmt:cmt + 1], in1=wb0,
                                    op0=ALU.add, op1=ALU.mult)
                            else:
                                tmp = pa1.tile([128, 512], F32, tag="tmp")
                                nc.vector.scalar_tensor_tensor(
                                    out=tmp, in0=pe,
                                    scalar=bet256_sb[:, cmt:cmt + 1], in1=wb1,
                                    op0=ALU.add, op1=ALU.mult)
                                nc.vector.reduce_sum(
                                    msum_f[:, cmt, q * 32:(q + 1) * 32],
                                    tmp.rearrange("p (f h o) -> p f h o", f=4, h=8),
                                    axis=AX.X)

                        for mt in range(8):
                            pe = paps.tile([128, 512], F32, tag="wave")
                            wave_mms(pe, mt)
                            e_ps.append(pe)
                            if mt == 1:
                                pw_b = papss.tile([128, 512], F32, tag="pw")
                                nc.tensor.matmul(pw_b, lhsT=ones_b[0:1, 0:128], rhs=w_sb,
                                                 start=True, stop=True)
                                if step == 0:
                                    nc.scalar.activation(wb0, pw_b, AF.Copy, scale=1.0 / 16)
                                    nc.vector.tensor_tensor(diag64, mask64_sb, wb0[0:64, :],
                                                            op=ALU.mult)
                                else:
                                    nc.scalar.activation(wb1, pw_b, AF.Copy, scale=1.0 / 4096)
                                    nc.scalar.activation(wb0, pw_b, AF.Copy, scale=1.0 / 16)
                            if mt >= 1:
                                for cmt in ([0, 1] if mt == 1 else [mt]):
                                    combine(cmt, e_ps[cmt])
                        if step == 1:
                            # msg_n half of M_sum: sum_o (w2/16) * msg_n
                            wb4 = wb0.rearrange("p (f h o) -> p f h o", f=4, h=8)
                            for j in range(8):
                                mt = 8 + j
                                base = msgn_sb[:, j, q * 64:(q + 1) * 64]
                                mn_bc = bass.AP(tensor=base.tensor, offset=base.offset,
                                                ap=[list(base.ap[0]), [16, 4], [0, 8], [1, 16]])
                                tmp2 = pa1.tile([128, 512], F32, tag="tmp2")
                                nc.gpsimd.tensor_tensor(
                                    tmp2.rearrange("p (f h o) -> p f h o", f=4, h=8),
                                    mn_bc, wb4, op=ALU.mult)
                                nc.vector.reduce_sum(
                                    msum_f[:, mt, q * 32:(q + 1) * 32],
                                    tmp2.rearrange("p (f h o) -> p f h o", f=4, h=8),
                                    axis=AX.X)
                        # interleave C0 blocks (keeps gsh streaming during A)
                        for j in c0_sched[q * 2 + step]:
                            emit_c0(j)
                nc.vector.tensor_scalar_mul(msum8, msum_f, 32.0)

        # ---------------- Phases B+C scope ----------------
        pbc = ctx.enter_context(tc.tile_pool(name="pbc", bufs=1))
        # S-GRU wih, e3m4, cached whole in SBUF: used by both C1 and C2
        gsi_all = pbc.tile([128, KC, 3 * D], FP8E3)

        def prefetch_gsi(j):
            nc.sync.dma_start(out=gsi_all[:, :, j * 512:(j + 1) * 512],
                              in_=gsi.ap()[:, j * 512:(j + 1) * 512]
                              .rearrange("(kc p) m -> p kc m", p=128))

        # ---------------- Phase B: human GRU (fp8 DoubleRow, row-major) ----------------
        with (
            tc.tile_pool(name="pbw", bufs=3) as pbw,
            tc.tile_pool(name="pbb", bufs=3) as pbb,
            tc.tile_pool(name="pb1", bufs=1) as pb1,
            tc.tile_pool(name="pbps", bufs=1, space="PSUM") as pbps,
            tc.tile_pool(name="pbps2", bufs=2, space="PSUM") as pbps2,
        ):
            NR = FPC * H  # 128 rows
            h_rm = pb1.tile([NR, D], BF16)
            nc.sync.dma_start(out=h_rm, in_=h_rmd.ap())
            pmat_sb = pb1.tile([NR, FPC], BF16)
            nc.sync.dma_start(out=pmat_sb, in_=pmatd.ap())
            hum_b = pb1.tile([NR, D], BF16)

            def gh_block(j, pt, use_i, use_h):
                """accumulate 2048*(gi and/or gh) for gate block j into psum
                pt, row-major [128 rows, 512 gates], fp8 DoubleRow."""
                ops = []
                if use_h:
                    wb_t = pbw.tile([128, KC, 512], FP8, tag="bwh")
                    nc.sync.dma_start(out=wb_t, in_=ghh.ap()[:, j * 512:(j + 1) * 512]
                                      .rearrange("(kc p) m -> p kc m", p=128))
                    bb = pbb.tile([1, 512], BF16, tag="bbh")
                    nc.sync.dma_start(out=bb, in_=ghhb.ap()[:, j * 512:(j + 1) * 512])
                    ops += [(wb_t, ht8_sb, k2) for k2 in range(8)] + [(bb, None, None)]
                if use_i:
                    wi_t = pbw.tile([128, KC, 512], FP8, tag="bwi")
                    nc.sync.dma_start(out=wi_t, in_=ghi.ap()[:, j * 512:(j + 1) * 512]
                                      .rearrange("(kc p) m -> p kc m", p=128))
                    bi = pbb.tile([1, 512], BF16, tag="bbi")
                    nc.sync.dma_start(out=bi, in_=ghib.ap()[:, j * 512:(j + 1) * 512])
                    ops += [(wi_t, msum8, k2) for k2 in range(8)] + [(bi, None, None)]
                for idx, (w, x, k2) in enumerate(ops):
                    st, sp = idx == 0, idx == len(ops) - 1
                    if x is None:
                        nc.tensor.matmul(pt, lhsT=ones_b[0:1, 0:128], rhs=w[0:1, :],
                                         start=st, stop=sp)
                    else:
                        nc.tensor.matmul(pt, lhsT=x[:, 2 * k2:2 * k2 + 2, :],
                                         rhs=w[:, 2 * k2:2 * k2 + 2, :],
                                         start=st, stop=sp, perf_mode=DR)

            for t in range(4):
                cols = slice(t * 512, (t + 1) * 512)
                # h-only group first: it has no msum dependency, so the PE can
                # chew on it while the q=3 M_sum combines drain on DVE
                p_hn = pbps.tile([NR, 512], F32, tag="phn")
                gh_block(8 + t, p_hn, False, True)
                p_r = pbps.tile([NR, 512], F32, tag="pr")
                gh_block(t, p_r, True, True)
                p_z = pbps.tile([NR, 512], F32, tag="pz")
                gh_block(4 + t, p_z, True, True)
                p_in = pbps.tile([NR, 512], F32, tag="pin")
                gh_block(8 + t, p_in, True, False)
                r_sb = pb1.tile([NR, 512], F32, tag="r")
                nc.scalar.activation(r_sb, p_r, AF.Sigmoid, scale=1.0 / 2048)
                z_sb = pb1.tile([NR, 512], F32, tag="z")
                nc.scalar.activation(z_sb, p_z, AF.Sigmoid, scale=1.0 / 2048)
                t1 = pb1.tile([NR, 512], F32, tag="t1")
                nc.vector.tensor_tensor(t1, r_sb, p_hn, op=ALU.mult)
                t2 = pb1.tile([NR, 512], F32, tag="r", name="t2")
                nc.vector.tensor_tensor(t2, t1, p_in, op=ALU.add)
                n_sb = pb1.tile([NR, 512], F32, tag="n")
                nc.scalar.activation(n_sb, t2, AF.Tanh, scale=1.0 / 2048)
                t3 = pb1.tile([NR, 512], F32, tag="t3")
                nc.vector.tensor_tensor(t3, h_rm[:, cols], n_sb, op=ALU.subtract)
                t4 = pb1.tile([NR, 512], F32, tag="t1", name="t4")
                nc.vector.tensor_tensor(t4, z_sb, t3, op=ALU.mult)
                nc.vector.tensor_tensor(hum_b[:, cols], n_sb, t4, op=ALU.add)
            for j in range(12):
                prefetch_gsi(j)
            # All_human^T chunks via PE: ah[c] = hum[:, c-chunk].T @ pmat
            for c in range(KC):
                pah = pbps2.tile([128, FPC], F32, tag="pah")
                nc.tensor.matmul(pah, lhsT=hum_b[:, c * 128:(c + 1) * 128], rhs=pmat_sb,
                                 start=True, stop=True)
                nc.scalar.activation(ah_sb[:, c, :], pah, AF.Copy, scale=2.0)

        # ---------------- Phase C: two S-node GRUs (wih e3m4 cached) ----------------
        with (
            tc.tile_pool(name="pc1", bufs=1) as pc1,
            tc.tile_pool(name="pcsm", bufs=1) as pcsm,
            tc.tile_pool(name="pcps", bufs=2, space="PSUM") as pcps,
            tc.tile_pool(name="pctps", bufs=2, space="PSUM") as pctps,
        ):
            gsib_sb = pc1.tile([1, 3 * D], BF16)
            nc.sync.dma_start(out=gsib_sb, in_=gsib.ap())
            sc4rm_sb = pc1.tile([FPC, D], F32)
            nc.sync.dma_start(out=sc4rm_sb, in_=sc4rmd.ap())
            sfrm_sb = pc1.tile([FPC, D], F32)
            nc.sync.dma_start(out=sfrm_sb, in_=sfrmd.ap())
            g1_sb = pc1.tile([16, 12, 512], BF16)
            g2_sb = pc1.tile([16, 12, 512], BF16)
            s1_sb = pc1.tile([16, D], BF16)
            s1t = pc1.tile([128, KC, 16], FP8E3)
            out32 = pc1.tile([FPC, D], F32)

            def sgru_wave(j, lhs_kc):
                PZ = pcps.tile([16, 512], F32, tag="PC")
                for kc in range(KC):
                    nc.tensor.matmul(PZ, lhsT=lhs_kc[:, kc, :],
                                     rhs=gsi_all[:, kc, j * 512:(j + 1) * 512],
                                     start=(kc == 0), stop=False)
                nc.tensor.matmul(PZ, lhsT=ones_b[0:1, 0:16],
                                 rhs=gsib_sb[0:1, j * 512:(j + 1) * 512],
                                 start=False, stop=True)
                return PZ

            # j order groups (r_t, z_t, n_t) so col-block t's elementwise can
            # start after its triple; transposes/output interleave under PE
            jorder = [0, 4, 8, 1, 5, 9, 2, 6, 10, 3, 7, 11]

            def s_combine(step, j, PZ):
                gx_sb = g1_sb if step == 0 else g2_sb
                gh_sb = gh1_sb if step == 0 else gh2_sb
                if j < 8:
                    nc.vector.scalar_tensor_tensor(out=gx_sb[:, j, :], in0=PZ,
                                                   scalar=1.0 / 128, in1=gh_sb[:, j, :],
                                                   op0=ALU.mult, op1=ALU.add)
                else:
                    nc.scalar.activation(gx_sb[:, j, :], PZ, AF.Copy, scale=1.0 / 128)

            def s_elem(step, t):
                cols = slice(t * 512, (t + 1) * 512)
                gx_sb = g1_sb if step == 0 else g2_sb
                gh_sb = gh1_sb if step == 0 else gh2_sb
                hsb = sc4rm_sb if step == 0 else sfrm_sb
                dst = s1_sb if step == 0 else out32
                z1 = pcsm.tile([16, 512], F32, tag="z1", name="z1")
                nc.scalar.activation(z1, gx_sb[:, 4 + t, :], AF.Sigmoid)
                r1 = pcsm.tile([16, 512], F32, tag="r1", name="r1")
                nc.scalar.activation(r1, gx_sb[:, t, :], AF.Sigmoid)
                u1 = pcsm.tile([16, 512], F32, tag="u1", name="u1")
                nc.vector.tensor_tensor(u1, r1, gh_sb[:, 8 + t, :], op=ALU.mult)
                u2 = pcsm.tile([16, 512], F32, tag="u2", name="u2")
                nc.vector.tensor_tensor(u2, u1, gx_sb[:, 8 + t, :], op=ALU.add)
                n1 = pcsm.tile([16, 512], F32, tag="n1", name="n1")
                nc.scalar.activation(n1, u2, AF.Tanh)
                u3 = pcsm.tile([16, 512], F32, tag="u3", name="u3")
                nc.vector.tensor_tensor(u3, hsb[:, cols], n1, op=ALU.subtract)
                u4 = pcsm.tile([16, 512], F32, tag="u4", name="u4")
                nc.vector.tensor_tensor(u4, z1, u3, op=ALU.mult)
                nc.vector.tensor_tensor(dst[:, cols], n1, u4, op=ALU.add)
                if step == 0:
                    # transpose finished s1 cols -> s1t (e3m4, 2*s1)
                    for c in range(4 * t, 4 * t + 4):
                        ptp = pctps.tile([128, 16], BF16, tag="tp", name="tp")
                        nc.tensor.transpose(ptp, s1_sb[:, c * 128:(c + 1) * 128], ident16)
                        nc.scalar.activation(s1t[:, c, :], ptp, AF.Copy, scale=2.0)
                else:
                    nc.sync.dma_start(out=outp.ap()[:, cols], in_=out32[:, cols])

            # step 1: gi1 = wih @ All_human + bih (psum = 128 * gi1), then s1
            for idx, j in enumerate(jorder):
                PZ = sgru_wave(j, ah_sb)
                s_combine(0, j, PZ)
                if idx % 3 == 2:
                    s_elem(0, idx // 3)
            # step 2: gi2 = wih @ s1 + bih; combine with gh2 -> out
            for idx, j in enumerate(jorder):
                PZ = sgru_wave(j, s1t)
                s_combine(1, j, PZ)
                if idx % 3 == 2:
                    s_elem(1, idx // 3)

    nc.compile()
    return nc


def _prep_in_maps(inputs):
    E = np.ascontiguousarray(inputs["H_O_edges"].reshape(NFRAMES, ROWS, D))
    On = inputs["O_nodes"].reshape(NFRAMES, O, D)
    Hn = inputs["H_nodes"].reshape(NFRAMES, H, D)
    Sc4 = inputs["S_node_C4"].reshape(NFRAMES, D)
    Sf = np.ascontiguousarray(inputs["final_S_node"].transpose(0, 2, 1)).reshape(NFRAMES, D)

    mask64 = np.zeros((64, 512), np.float32)
    for f in range(4):
        for o in range(O):
            for h in range(H):
                mask64[f * 16 + o, f * 128 + h * 16 + o] = 64.0

    shared = {
        "wcat": q8(np.concatenate([inputs["We"], inputs["Wl1"]], axis=0).T, 64.0),
        "bl1t": np.ascontiguousarray(inputs["bl1"].reshape(8, 128).T).astype(np.float32),
        "bet64": np.ascontiguousarray(inputs["be"].reshape(8, 128).T).astype(np.float32) * 64.0,
        "bet256": np.ascontiguousarray(inputs["be"].reshape(8, 128).T).astype(np.float32) * 256.0,
        "mask64": mask64.astype(NB),
        "pmat": np.ascontiguousarray(np.kron(np.eye(FPC), np.ones((H, 1))) / H).astype(NB),
        "wnt": q8(inputs["Wn"].T, 64.0),
        "wnb64": (inputs["bn"][None, :] * 64.0).astype(NB),
        "wl2": np.ascontiguousarray(inputs["Wl2"][0].reshape(8, 128).T).astype(NB),
        "ghi": q8(inputs["gh_wih"].T, 64.0),
        "ghib": (inputs["gh_bih"][None, :] * 2048.0).astype(NB),
        "ghh": q8(inputs["gh_whh"].T, 64.0),
        "ghhb": (inputs["gh_bhh"][None, :] * 2048.0).astype(NB),
        "gsi": qe3(np.ascontiguousarray(inputs["gs_wih"].T), 64.0),
        "gsib": (inputs["gs_bih"][None, :] * 128.0).astype(NB),
        "gsh": np.ascontiguousarray(inputs["gs_whh"].T).astype(NB),
        "gshb": inputs["gs_bhh"][None, :].astype(NB),
    }

    in_maps = []
    for c in range(NCORES):
        fr = slice(c * FPC, (c + 1) * FPC)
        Ec = E[fr]  # [16, 128, 2048]
        e0t = q8(Ec.reshape(NQ, 4, ROWS, D).transpose(0, 3, 1, 2).reshape(NQ, D, 512), 1.0)
        m = dict(shared)
        m.update({
            "e0t": e0t,
            "ot": q8(On[fr].reshape(FPC * O, D).T, 1.0),
            "ht8": q8(Hn[fr].reshape(FPC * H, D).T, 32.0),
            "h_rm": np.ascontiguousarray(Hn[fr].reshape(FPC * H, D)).astype(NB),
            "scsf": np.ascontiguousarray(np.concatenate(
                [Sc4[fr].T, np.zeros((D, FPC), np.float32), Sf[fr].T], axis=1)).astype(NB),
            "sc4rm": np.ascontiguousarray(Sc4[fr]).astype(np.float32),
            "sfrm": np.ascontiguousarray(Sf[fr]).astype(np.float32),
        })
        in_maps.append(m)
    return in_maps


LAST_RESULT = None


def kernel(**inputs):
    global LAST_RESULT
    if "nc" not in _CACHE:
        _CACHE["nc"] = _build_nc()
    nc = _CACHE["nc"]
    in_maps = _prep_in_maps(inputs)
    trace = os.environ.get("KERNEL_TRACE", "0") == "1"
    res = bass_utils.run_bass_kernel_spmd(
        nc, in_maps, core_ids=list(range(NCORES)), trace=trace)
    LAST_RESULT = res
    out = np.concatenate([res.results[c]["outp"] for c in range(NCORES)], axis=0)
    return np.ascontiguousarray(out.reshape(B, F, D)).astype(np.float32)


# revision 34
# speedup vs baseline: 1.0030x; 1.0030x over previous
"""Trainium2 Bass kernel for nn_Graph_Enhance_model (GNN message passing).

Self-contained: hardcodes shapes B=4,F=32,H=8,O=16,D=2048, 8 cores.
Data-parallel over the 128 (b,f) frames: 16 frames per core.

v2: fp8e4m3 DoubleRow for the edge waves + human GRU (validated: final
rel_err ~2.4e-3 vs 2e-2 budget), algebraic rank-trick for propagation
step 2 (edges after step 1 are w * [msg_e | bcast(msg_n)]; the msg_n
half is rank-16 per frame, so step 2 contracts K=1024 + a rank-16
correction), S-node GRU kept bf16, its input-independent half (whh @
[sc4|sf]) interleaved into Phase A so its weight DMA hides under the
compute-bound wave phase.

Scale conventions (power-of-2, folded into host tensors / activation
scale):
  wcat = 64*[We|Wl1]^T fp8      wave psum (step0) = 64 * true
  e0t  = edges fp8 (x1)         wave psum (step1) = 256 * true
  um1t = 4 * (w1 x msg_e1) fp8  c_sb = 64 * (msg_n @ W_R^T) bf16
  diag64 = 4 * w1 bf16 (mask64 host = 64, times wb0 = w/16)
  ghi/ghh = 64*w^T fp8, msum8 = 32*msum, ht8 = 32*h^T -> psum = 2048x
"""

import os
import sys

for _p in ("/opt/trn_rl_repo", "/opt/pypackages"):
    if _p not in sys.path and os.path.isdir(_p):
        sys.path.append(_p)

import numpy as np
import ml_dtypes

import concourse.bass as bass
import concourse.bacc as bacc
import concourse.tile as tile
import concourse.mybir as mybir
from concourse import bass_utils
from concourse.masks import make_identity

BF16 = mybir.dt.bfloat16
FP8 = mybir.dt.float8e4
F32 = mybir.dt.float32
AF = mybir.ActivationFunctionType
ALU = mybir.AluOpType
AX = mybir.AxisListType
DR = mybir.MatmulPerfMode.DoubleRow

NB = ml_dtypes.bfloat16
N8 = ml_dtypes.float8_e4m3fn
NE3 = ml_dtypes.float8_e3m4
FP8E3 = mybir.dt.float8e3

B, F, H, O, D = 4, 32, 8, 16, 2048
NFRAMES = B * F          # 128
NCORES = 8
FPC = NFRAMES // NCORES  # 16 frames per core
ROWS = H * O             # 128 rows per frame
KC = D // 128            # 16 K-chunks
NQ = FPC // 4            # 4 quads of 4 frames

_CACHE = {}


def q8(x, scale):
    return np.clip(np.asarray(x, np.float32) * scale, -240, 240).astype(N8)


def qe3(x, scale):
    return np.clip(np.asarray(x, np.float32) * scale, -15.5, 15.5).astype(NE3)


def _build_nc():
    nc = bacc.Bacc("TRN2", target_bir_lowering=False, debug=False, num_devices=NCORES)

    def din(name, shape, dt):
        return nc.dram_tensor(name, shape, dt, kind="ExternalInput")

    e0t = din("e0t", [NQ, D, 512], FP8)
    ot = din("ot", [D, FPC * O], FP8)
    wnt = din("wnt", [D, D // 2], FP8)
    wnb64 = din("wnb64", [1, D // 2], BF16)
    wcat = din("wcat", [D, D], FP8)
    bl1td = din("bl1t", [128, 8], F32)
    bet64d = din("bet64", [128, 8], F32)
    bet256d = din("bet256", [128, 8], F32)
    wl2 = din("wl2", [128, 8], BF16)
    mask64d = din("mask64", [64, 512], BF16)
    ht8d = din("ht8", [D, FPC * H], FP8)
    h_rmd = din("h_rm", [FPC * H, D], BF16)
    pmatd = din("pmat", [FPC * H, FPC], BF16)
    ghi = din("ghi", [D, 3 * D], FP8)
    ghib = din("ghib", [1, 3 * D], BF16)
    ghh = din("ghh", [D, 3 * D], FP8)
    ghhb = din("ghhb", [1, 3 * D], BF16)
    scsf = din("scsf", [D, 3 * FPC], BF16)
    sc4rmd = din("sc4rm", [FPC, D], F32)
    sfrmd = din("sfrm", [FPC, D], F32)
    gsi = din("gsi", [D, 3 * D], FP8E3)
    gsib = din("gsib", [1, 3 * D], BF16)
    gsh = din("gsh", [D, 3 * D], BF16)
    gshb = din("gshb", [1, 3 * D], BF16)
    outp = nc.dram_tensor("outp", [FPC, D], F32, kind="ExternalOutput")

    from contextlib import ExitStack

    with tile.TileContext(nc) as tc, ExitStack() as ctx:
        glob = ctx.enter_context(tc.tile_pool(name="glob", bufs=1))

        ones_b = glob.tile([1, 512], BF16)
        nc.vector.memset(ones_b, 1.0)
        ident16 = glob.tile([16, 16], BF16)
        make_identity(nc, ident16)

        wl2_sb = glob.tile([128, 8], BF16)
        bl1t_sb = glob.tile([128, 8], F32)
        bet64_sb = glob.tile([128, 8], F32)
        bet256_sb = glob.tile([128, 8], F32)
        mask64_sb = glob.tile([64, 512], BF16)

        msum8 = glob.tile([128, KC, FPC * H], FP8)      # 32 * msum
        ht8_sb = glob.tile([128, KC, FPC * H], FP8)     # 32 * h^T
        scsf_b = glob.tile([128, KC, 3 * FPC], BF16)    # [S_C4^T | 0 | S_f^T]
        gh1_sb = glob.tile([16, 12, 512], BF16)         # whh@sc4 + bhh
        gh2_sb = glob.tile([16, 12, 512], BF16)         # whh@sf + bhh
        ah_sb = glob.tile([128, KC, FPC], FP8E3)        # 2 * All_human^T

        with (
            tc.tile_pool(name="pwcat", bufs=1) as pwcat,
            tc.tile_pool(name="pa", bufs=2) as pa,
            tc.tile_pool(name="pa1", bufs=1) as pa1,
            tc.tile_pool(name="pc0w", bufs=2) as pc0w,
        ):
            wcat_sb = pwcat.tile([128, KC, D], FP8)
            msgn_sb = pa1.tile([128, 8, FPC * O], BF16)   # true msg_n^T (+bn)
            c_sb = pa1.tile([64, NQ, D], BF16)            # 64 * msg_n @ W_R^T
            msum_f = pa1.tile([128, KC, FPC * H], F32)    # true M_sum2^T

            # ---------------- Phase 0: msg_n^T = Wn @ O^T + bn (fp8 DR) ----------------
            with (
                tc.tile_pool(name="p0", bufs=1) as p0,
                tc.tile_pool(name="p0ps", bufs=4, space="PSUM") as p0ps,
            ):
                # DMA issue order = critical-path order: phase-0 inputs, then
                # first edge quad, then wcat (chunked so waves start early),
                # then everything else.
                ot_sb = p0.tile([128, KC, FPC * O], FP8)
                nc.sync.dma_start(out=ot_sb, in_=ot.ap().rearrange("(kc p) n -> p kc n", p=128))
                wn_sb = p0.tile([128, KC, D // 2], FP8)
                nc.sync.dma_start(out=wn_sb, in_=wnt.ap().rearrange("(kc p) m -> p kc m", p=128))
                wnb_sb = p0.tile([1, D // 2], BF16)
                nc.sync.dma_start(out=wnb_sb, in_=wnb64.ap())
                msgn8 = p0.tile([128, 8, FPC * O], FP8)   # 16 * msg_n
                xqs = {}
                for q in range(2):
                    xqs[q] = pa.tile([128, KC, 512], FP8, tag="xq", name="xq")
                    nc.sync.dma_start(out=xqs[q],
                                      in_=e0t.ap()[q].rearrange("(kc p) n -> p kc n", p=128))
                for k2 in range(8):
                    nc.sync.dma_start(
                        out=wcat_sb[:, 2 * k2:2 * k2 + 2, :],
                        in_=wcat.ap()[256 * k2:256 * (k2 + 1), :]
                        .rearrange("(kc p) m -> p kc m", p=128))
                nc.sync.dma_start(out=wl2_sb, in_=wl2.ap())
                nc.sync.dma_start(out=bl1t_sb, in_=bl1td.ap())
                nc.sync.dma_start(out=bet64_sb, in_=bet64d.ap())
                nc.sync.dma_start(out=bet256_sb, in_=bet256d.ap())
                nc.sync.dma_start(out=mask64_sb, in_=mask64d.ap())
                nc.sync.dma_start(out=ht8_sb, in_=ht8d.ap().rearrange("(kc p) n -> p kc n", p=128))
                nc.sync.dma_start(out=scsf_b, in_=scsf.ap().rearrange("(kc p) n -> p kc n", p=128))
                for mt in range(8):
                    pm = p0ps.tile([128, FPC * O], F32, tag="pm")
                    for k2 in range(8):
                        nc.tensor.matmul(pm, lhsT=wn_sb[:, 2 * k2:2 * k2 + 2, mt * 128:(mt + 1) * 128],
                                         rhs=ot_sb[:, 2 * k2:2 * k2 + 2, :],
                                         start=(k2 == 0), stop=False, perf_mode=DR)
                    nc.tensor.matmul(pm, lhsT=wnb_sb[0:1, mt * 128:(mt + 1) * 128],
                                     rhs=ones_b[0:1, 0:FPC * O], start=False, stop=True)
                    nc.scalar.activation(msgn_sb[:, mt, :], pm, AF.Copy, scale=1.0 / 64)
                    nc.scalar.activation(msgn8[:, mt, :], pm, AF.Copy, scale=1.0 / 4)

                # c = msg_n @ [We_R | Wl1_R]^T  (per-q layout, 64*c in bf16)
                for q in range(NQ):
                    for mtile in range(4):
                        pcp = p0ps.tile([64, 512], F32, tag="pc")
                        for k2 in range(4):
                            nc.tensor.matmul(
                                pcp,
                                lhsT=msgn8[:, 2 * k2:2 * k2 + 2, q * 64:(q + 1) * 64],
                                rhs=wcat_sb[:, 8 + 2 * k2:8 + 2 * k2 + 2,
                                            mtile * 512:(mtile + 1) * 512],
                                start=(k2 == 0), stop=(k2 == 3), perf_mode=DR)
                        nc.scalar.activation(c_sb[:, q, mtile * 512:(mtile + 1) * 512],
                                             pcp, AF.Copy, scale=1.0 / 16)

            # ---------------- Phase A: 2 propagation steps + interleaved C0 ----------------
            # C0 = S-GRU whh @ [sc4 | sf] (input-independent), spread across
            # slots; back-loaded so its PE work covers the q=3 DVE tail.
            c0_sched = [[], [0], [1], [2], [3, 4], [5, 6], [7, 8], [9, 10, 11]]

            def emit_c0(j):
                wsh = pc0w.tile([128, KC, 512], BF16, tag="wsh")
                nc.sync.dma_start(out=wsh, in_=gsh.ap()[:, j * 512:(j + 1) * 512]
                                  .rearrange("(kc p) m -> p kc m", p=128))
                bsh = pc0w.tile([1, 512], BF16, tag="bsh")
                nc.sync.dma_start(out=bsh, in_=gshb.ap()[:, j * 512:(j + 1) * 512])
                PH = pc0ps.tile([48, 512], F32, tag="PH")
                for kc in range(KC):
                    nc.tensor.matmul(PH, lhsT=scsf_b[:, kc, :], rhs=wsh[:, kc, :],
                                     start=(kc == 0), stop=False)
                nc.tensor.matmul(PH, lhsT=ones_b[0:1, 0:48], rhs=bsh[0:1, :],
                                 start=False, stop=True)
                nc.scalar.copy(gh1_sb[:, j, :], PH[0:16, :])
                nc.scalar.copy(gh2_sb[:, j, :], PH[32:48, :])

            with tc.tile_pool(name="paps", bufs=4, space="PSUM") as paps, \
                 tc.tile_pool(name="papss", bufs=1, space="PSUM") as papss, \
                 tc.tile_pool(name="pc0ps", bufs=1, space="PSUM") as pc0ps:
                for q in range(NQ):
                    if q in xqs:
                        xq = xqs[q]
                    else:
                        xq = pa.tile([128, KC, 512], FP8, tag="xq")
                        nc.sync.dma_start(out=xq, in_=e0t.ap()[q].rearrange("(kc p) n -> p kc n", p=128))
                    um1t = pa1.tile([128, 8, 512], FP8, tag="um1t")
                    diag64 = pa1.tile([64, 512], BF16, tag="diag")
                    for step in range(2):
                        psc = 1.0 / 64 if step == 0 else 1.0 / 256

                        def wave_mms(pt, mcol):
                            """full K-contraction for output cols mcol*128
                            into psum pt (start..stop)."""
                            if step == 0:
                                for k2 in range(8):
                                    nc.tensor.matmul(
                                        pt, lhsT=wcat_sb[:, 2 * k2:2 * k2 + 2,
                                                         mcol * 128:(mcol + 1) * 128],
                                        rhs=xq[:, 2 * k2:2 * k2 + 2, :],
                                        start=(k2 == 0), stop=(k2 == 7), perf_mode=DR)
                            else:
                                for k2 in range(4):
                                    nc.tensor.matmul(
                                        pt, lhsT=wcat_sb[:, 2 * k2:2 * k2 + 2,
                                                         mcol * 128:(mcol + 1) * 128],
                                        rhs=um1t[:, 2 * k2:2 * k2 + 2, :],
                                        start=(k2 == 0), stop=False, perf_mode=DR)
                                nc.tensor.matmul(
                                    pt, lhsT=c_sb[:, q, mcol * 128:(mcol + 1) * 128],
                                    rhs=diag64, start=False, stop=True)

                        # --- a-wave: relu(X @ Wl1^T + bl1), transposed ---
                        relu_sb = pa1.tile([128, 8, 512], BF16, tag="relu")
                        for mt in range(8, 16):
                            pw_a = paps.tile([128, 512], F32, tag="wave")
                            wave_mms(pw_a, mt)
                            nc.scalar.activation(relu_sb[:, mt - 8, :], pw_a, AF.Relu,
                                                 bias=bl1t_sb[:, mt - 8:mt - 7], scale=psc)
                        # --- logits + softmax over o (groups of 16) ---
                        pl = papss.tile([1, 512], F32, tag="pl")
                        for kc2 in range(8):
                            nc.tensor.matmul(pl, lhsT=wl2_sb[:, kc2:kc2 + 1],
                                             rhs=relu_sb[:, kc2, :], start=(kc2 == 0), stop=(kc2 == 7))
                        pl3 = pl.rearrange("o (g i) -> o g i", i=16)
                        mx = pa1.tile([1, 32], F32, tag="mx")
                        nc.vector.reduce_max(mx, pl3, axis=AX.X)
                        sub = pa1.tile([1, 512], F32, tag="sub")
                        nc.vector.tensor_tensor(sub.rearrange("o (g i) -> o g i", i=16), pl3,
                                                mx.broadcast_to((1, 32, 16)), op=ALU.subtract)
                        nc.scalar.activation(sub, sub, AF.Exp)
                        ex3 = sub.rearrange("o (g i) -> o g i", i=16)
                        sm = pa1.tile([1, 32], F32, tag="sm")
                        nc.vector.reduce_sum(sm, ex3, axis=AX.X)
                        rs = pa1.tile([1, 32], F32, tag="rs")
                        nc.vector.reciprocal(rs, sm)
                        w_sb = pa1.tile([1, 512], BF16, tag="w")
                        nc.vector.tensor_tensor(w_sb.rearrange("o (g i) -> o g i", i=16), ex3,
                                                rs.broadcast_to((1, 32, 16)), op=ALU.mult)
                        # --- msg_e wave; w-broadcast emitted after 2 groups ---
                        e_ps = []
                        wb0 = pa1.tile([128, 512], F32, tag="wb0")
                        wb1 = pa1.tile([128, 512], F32, tag="wb1")

                        def combine(cmt, pe):
                            if step == 0:
                                nc.vector.scalar_tensor_tensor(
                                    out=um1t[:, cmt, :], in0=pe,
                                    scalar=bet64_sb[:, cmt:cmt + 1], in1=wb0,
                                    op0=ALU.add, op1=ALU.mult)
                            else:
                                tmp = pa1.tile([128, 512], F32, tag="tmp")
                                nc.vector.scalar_tensor_tensor(
                                    out=tmp, in0=pe,
                                    scalar=bet256_sb[:, cmt:cmt + 1], in1=wb1,
                                    op0=ALU.add, op1=ALU.mult)
                                nc.vector.reduce_sum(
                                    msum_f[:, cmt, q * 32:(q + 1) * 32],
                                    tmp.rearrange("p (f h o) -> p f h o", f=4, h=8),
                                    axis=AX.X)

                        for mt in range(8):
                            pe = paps.tile([128, 512], F32, tag="wave")
                            wave_mms(pe, mt)
                            e_ps.append(pe)
                            if mt == 1:
                                pw_b = papss.tile([128, 512], F32, tag="pw")
                                nc.tensor.matmul(pw_b, lhsT=ones_b[0:1, 0:128], rhs=w_sb,
                                                 start=True, stop=True)
                                if step == 0:
                                    nc.scalar.activation(wb0, pw_b, AF.Copy, scale=1.0 / 16)
                                    nc.vector.tensor_tensor(diag64, mask64_sb, wb0[0:64, :],
                                                            op=ALU.mult)
                                else:
                                    nc.scalar.activation(wb1, pw_b, AF.Copy, scale=1.0 / 4096)
                                    nc.scalar.activation(wb0, pw_b, AF.Copy, scale=1.0 / 16)
                            if mt >= 1:
                                for cmt in ([0, 1] if mt == 1 else [mt]):
                                    combine(cmt, e_ps[cmt])
                        if step == 1:
                            # msg_n half of M_sum: sum_o (w2/16) * msg_n
                            wb4 = wb0.rearrange("p (f h o) -> p f h o", f=4, h=8)
                            for j in range(8):
                                mt = 8 + j
                                base = msgn_sb[:, j, q * 64:(q + 1) * 64]
                                mn_bc = bass.AP(tensor=base.tensor, offset=base.offset,
                                                ap=[list(base.ap[0]), [16, 4], [0, 8], [1, 16]])
                                tmp2 = pa1.tile([128, 512], F32, tag="tmp2")
                                nc.gpsimd.tensor_tensor(
                                    tmp2.rearrange("p (f h o) -> p f h o", f=4, h=8),
                                    mn_bc, wb4, op=ALU.mult)
                                nc.vector.reduce_sum(
                                    msum_f[:, mt, q * 32:(q + 1) * 32],
                                    tmp2.rearrange("p (f h o) -> p f h o", f=4, h=8),
                                    axis=AX.X)
                        # interleave C0 blocks (keeps gsh streaming during A)
                        for j in c0_sched[q * 2 + step]:
                            emit_c0(j)
                nc.vector.tensor_scalar_mul(msum8, msum_f, 32.0)

        # ---------------- Phases B+C scope ----------------
        pbc = ctx.enter_context(tc.tile_pool(name="pbc", bufs=1))
        # S-GRU wih, e3m4, cached whole in SBUF: used by both C1 and C2
        gsi_all = pbc.tile([128, KC, 3 * D], FP8E3)

        def prefetch_gsi(j):
            nc.sync.dma_start(out=gsi_all[:, :, j * 512:(j + 1) * 512],
                              in_=gsi.ap()[:, j * 512:(j + 1) * 512]
                              .rearrange("(kc p) m -> p kc m", p=128))

        # ---------------- Phase B: human GRU (fp8 DoubleRow, row-major) ----------------
        with (
            tc.tile_pool(name="pbw", bufs=3) as pbw,
            tc.tile_pool(name="pbb", bufs=3) as pbb,
            tc.tile_pool(name="pb1", bufs=1) as pb1,
            tc.tile_pool(name="pbps", bufs=1, space="PSUM") as pbps,
            tc.tile_pool(name="pbps2", bufs=2, space="PSUM") as pbps2,
        ):
            NR = FPC * H  # 128 rows
            h_rm = pb1.tile([NR, D], BF16)
            nc.sync.dma_start(out=h_rm, in_=h_rmd.ap())
            pmat_sb = pb1.tile([NR, FPC], BF16)
            nc.sync.dma_start(out=pmat_sb, in_=pmatd.ap())
            hum_b = pb1.tile([NR, D], BF16)

            def gh_block(j, pt, use_i, use_h):
                """accumulate 2048*(gi and/or gh) for gate block j into psum
                pt, row-major [128 rows, 512 gates], fp8 DoubleRow."""
                ops = []
                if use_h:
                    wb_t = pbw.tile([128, KC, 512], FP8, tag="bwh")
                    nc.sync.dma_start(out=wb_t, in_=ghh.ap()[:, j * 512:(j + 1) * 512]
                                      .rearrange("(kc p) m -> p kc m", p=128))
                    bb = pbb.tile([1, 512], BF16, tag="bbh")
                    nc.sync.dma_start(out=bb, in_=ghhb.ap()[:, j * 512:(j + 1) * 512])
                    ops += [(wb_t, ht8_sb, k2) for k2 in range(8)] + [(bb, None, None)]
                if use_i:
                    wi_t = pbw.tile([128, KC, 512], FP8, tag="bwi")
                    nc.sync.dma_start(out=wi_t, in_=ghi.ap()[:, j * 512:(j + 1) * 512]
                                      .rearrange("(kc p) m -> p kc m", p=128))
                    bi = pbb.tile([1, 512], BF16, tag="bbi")
                    nc.sync.dma_start(out=bi, in_=ghib.ap()[:, j * 512:(j + 1) * 512])
                    ops += [(wi_t, msum8, k2) for k2 in range(8)] + [(bi, None, None)]
                for idx, (w, x, k2) in enumerate(ops):
                    st, sp = idx == 0, idx == len(ops) - 1
                    if x is None:
                        nc.tensor.matmul(pt, lhsT=ones_b[0:1, 0:128], rhs=w[0:1, :],
                                         start=st, stop=sp)
                    else:
                        nc.tensor.matmul(pt, lhsT=x[:, 2 * k2:2 * k2 + 2, :],
                                         rhs=w[:, 2 * k2:2 * k2 + 2, :],
                                         start=st, stop=sp, perf_mode=DR)

            for t in range(4):
                cols = slice(t * 512, (t + 1) * 512)
                # h-only group first: it has no msum dependency, so the PE can
                # chew on it while the q=3 M_sum combines drain on DVE
                p_hn = pbps.tile([NR, 512], F32, tag="phn")
                gh_block(8 + t, p_hn, False, True)
                p_r = pbps.tile([NR, 512], F32, tag="pr")
                gh_block(t, p_r, True, True)
                p_z = pbps.tile([NR, 512], F32, tag="pz")
                gh_block(4 + t, p_z, True, True)
                p_in = pbps.tile([NR, 512], F32, tag="pin")
                gh_block(8 + t, p_in, True, False)
                for j in (3 * t, 3 * t + 1, 3 * t + 2):
                    prefetch_gsi(j)
                r_sb = pb1.tile([NR, 512], F32, tag="r")
                nc.scalar.activation(r_sb, p_r, AF.Sigmoid, scale=1.0 / 2048)
                z_sb = pb1.tile([NR, 512], F32, tag="z")
                nc.scalar.activation(z_sb, p_z, AF.Sigmoid, scale=1.0 / 2048)
                t1 = pb1.tile([NR, 512], F32, tag="t1")
                nc.vector.tensor_tensor(t1, r_sb, p_hn, op=ALU.mult)
                t2 = pb1.tile([NR, 512], F32, tag="r", name="t2")
                nc.vector.tensor_tensor(t2, t1, p_in, op=ALU.add)
                n_sb = pb1.tile([NR, 512], F32, tag="n")
                nc.scalar.activation(n_sb, t2, AF.Tanh, scale=1.0 / 2048)
                t3 = pb1.tile([NR, 512], F32, tag="t3")
                nc.vector.tensor_tensor(t3, h_rm[:, cols], n_sb, op=ALU.subtract)
                t4 = pb1.tile([NR, 512], F32, tag="t1", name="t4")
                nc.vector.tensor_tensor(t4, z_sb, t3, op=ALU.mult)
                nc.vector.tensor_tensor(hum_b[:, cols], n_sb, t4, op=ALU.add)
            # All_human^T chunks via PE: ah[c] = hum[:, c-chunk].T @ pmat
            for c in range(KC):
                pah = pbps2.tile([128, FPC], F32, tag="pah")
                nc.tensor.matmul(pah, lhsT=hum_b[:, c * 128:(c + 1) * 128], rhs=pmat_sb,
                                 start=True, stop=True)
                nc.scalar.activation(ah_sb[:, c, :], pah, AF.Copy, scale=2.0)

        # ---------------- Phase C: two S-node GRUs (wih e3m4 cached) ----------------
        with (
            tc.tile_pool(name="pc1", bufs=1) as pc1,
            tc.tile_pool(name="pcsm", bufs=1) as pcsm,
            tc.tile_pool(name="pcps", bufs=2, space="PSUM") as pcps,
            tc.tile_pool(name="pctps", bufs=2, space="PSUM") as pctps,
        ):
            gsib_sb = pc1.tile([1, 3 * D], BF16)
            nc.sync.dma_start(out=gsib_sb, in_=gsib.ap())
            sc4rm_sb = pc1.tile([FPC, D], F32)
            nc.sync.dma_start(out=sc4rm_sb, in_=sc4rmd.ap())
            sfrm_sb = pc1.tile([FPC, D], F32)
            nc.sync.dma_start(out=sfrm_sb, in_=sfrmd.ap())
            g1_sb = pc1.tile([16, 12, 512], BF16)
            g2_sb = pc1.tile([16, 12, 512], BF16)
            s1_sb = pc1.tile([16, D], BF16)
            s1t = pc1.tile([128, KC, 16], FP8E3)
            out32 = pc1.tile([FPC, D], F32)

            def sgru_wave(j, lhs_kc):
                PZ = pcps.tile([16, 512], F32, tag="PC")
                for kc in range(KC):
                    nc.tensor.matmul(PZ, lhsT=lhs_kc[:, kc, :],
                                     rhs=gsi_all[:, kc, j * 512:(j + 1) * 512],
                                     start=(kc == 0), stop=False)
                nc.tensor.matmul(PZ, lhsT=ones_b[0:1, 0:16],
                                 rhs=gsib_sb[0:1, j * 512:(j + 1) * 512],
                                 start=False, stop=True)
                return PZ

            # j order groups (r_t, z_t, n_t) so col-block t's elementwise can
            # start after its triple; transposes/output interleave under PE
            jorder = [0, 4, 8, 1, 5, 9, 2, 6, 10, 3, 7, 11]

            def s_combine(step, j, PZ):
                gx_sb = g1_sb if step == 0 else g2_sb
                gh_sb = gh1_sb if step == 0 else gh2_sb
                if j < 8:
                    nc.vector.scalar_tensor_tensor(out=gx_sb[:, j, :], in0=PZ,
                                                   scalar=1.0 / 128, in1=gh_sb[:, j, :],
                                                   op0=ALU.mult, op1=ALU.add)
                else:
                    nc.scalar.activation(gx_sb[:, j, :], PZ, AF.Copy, scale=1.0 / 128)

            def s_elem(step, t):
                cols = slice(t * 512, (t + 1) * 512)
                gx_sb = g1_sb if step == 0 else g2_sb
                gh_sb = gh1_sb if step == 0 else gh2_sb
                hsb = sc4rm_sb if step == 0 else sfrm_sb
                dst = s1_sb if step == 0 else out32
                z1 = pcsm.tile([16, 512], F32, tag="z1", name="z1")
                nc.scalar.activation(z1, gx_sb[:, 4 + t, :], AF.Sigmoid)
                r1 = pcsm.tile([16, 512], F32, tag="r1", name="r1")
                nc.scalar.activation(r1, gx_sb[:, t, :], AF.Sigmoid)
                u1 = pcsm.tile([16, 512], F32, tag="u1", name="u1")
                nc.vector.tensor_tensor(u1, r1, gh_sb[:, 8 + t, :], op=ALU.mult)
                u2 = pcsm.tile([16, 512], F32, tag="u2", name="u2")
                nc.vector.tensor_tensor(u2, u1, gx_sb[:, 8 + t, :], op=ALU.add)
                n1 = pcsm.tile([16, 512], F32, tag="n1", name="n1")
                nc.scalar.activation(n1, u2, AF.Tanh)
                u3 = pcsm.tile([16, 512], F32, tag="u3", name="u3")
                nc.vector.tensor_tensor(u3, hsb[:, cols], n1, op=ALU.subtract)
                u4 = pcsm.tile([16, 512], F32, tag="u4", name="u4")
                nc.vector.tensor_tensor(u4, z1, u3, op=ALU.mult)
                nc.vector.tensor_tensor(dst[:, cols], n1, u4, op=ALU.add)
                if step == 0:
                    # transpose finished s1 cols -> s1t (e3m4, 2*s1)
                    for c in range(4 * t, 4 * t + 4):
                        ptp = pctps.tile([128, 16], BF16, tag="tp", name="tp")
                        nc.tensor.transpose(ptp, s1_sb[:, c * 128:(c + 1) * 128], ident16)
                        nc.scalar.activation(s1t[:, c, :], ptp, AF.Copy, scale=2.0)
                else:
                    nc.sync.dma_start(out=outp.ap()[:, cols], in_=out32[:, cols])

            # step 1: gi1 = wih @ All_human + bih (psum = 128 * gi1), then s1
            for idx, j in enumerate(jorder):
                PZ = sgru_wave(j, ah_sb)
                s_combine(0, j, PZ)
                if idx % 3 == 2:
                    s_elem(0, idx // 3)
            # step 2: gi2 = wih @ s1 + bih; combine with gh2 -> out
            for idx, j in enumerate(jorder):
                PZ = sgru_wave(j, s1t)
                s_combine(1, j, PZ)
                if idx % 3 == 2:
                    s_elem(1, idx // 3)

    nc.compile()
    return nc


def _prep_in_maps(inputs):
    E = np.ascontiguousarray(inputs["H_O_edges"].reshape(NFRAMES, ROWS, D))
    On = inputs["O_nodes"].reshape(NFRAMES, O, D)
    Hn = inputs["H_nodes"].reshape(NFRAMES, H, D)
    Sc4 = inputs["S_node_C4"].reshape(NFRAMES, D)
    Sf = np.ascontiguousarray(inputs["final_S_node"].transpose(0, 2, 1)).reshape(NFRAMES, D)

    mask64 = np.zeros((64, 512), np.float32)
    for f in range(4):
        for o in range(O):
            for h in range(H):
                mask64[f * 16 + o, f * 128 + h * 16 + o] = 64.0

    shared = {
        "wcat": q8(np.concatenate([inputs["We"], inputs["Wl1"]], axis=0).T, 64.0),
        "bl1t": np.ascontiguousarray(inputs["bl1"].reshape(8, 128).T).astype(np.float32),
        "bet64": np.ascontiguousarray(inputs["be"].reshape(8, 128).T).astype(np.float32) * 64.0,
        "bet256": np.ascontiguousarray(inputs["be"].reshape(8, 128).T).astype(np.float32) * 256.0,
        "mask64": mask64.astype(NB),
        "pmat": np.ascontiguousarray(np.kron(np.eye(FPC), np.ones((H, 1))) / H).astype(NB),
        "wnt": q8(inputs["Wn"].T, 64.0),
        "wnb64": (inputs["bn"][None, :] * 64.0).astype(NB),
        "wl2": np.ascontiguousarray(inputs["Wl2"][0].reshape(8, 128).T).astype(NB),
        "ghi": q8(inputs["gh_wih"].T, 64.0),
        "ghib": (inputs["gh_bih"][None, :] * 2048.0).astype(NB),
        "ghh": q8(inputs["gh_whh"].T, 64.0),
        "ghhb": (inputs["gh_bhh"][None, :] * 2048.0).astype(NB),
        "gsi": qe3(np.ascontiguousarray(inputs["gs_wih"].T), 64.0),
        "gsib": (inputs["gs_bih"][None, :] * 128.0).astype(NB),
        "gsh": np.ascontiguousarray(inputs["gs_whh"].T).astype(NB),
        "gshb": inputs["gs_bhh"][None, :].astype(NB),
    }

    in_maps = []
    for c in range(NCORES):
        fr = slice(c * FPC, (c + 1) * FPC)
        Ec = E[fr]  # [16, 128, 2048]
        e0t = q8(Ec.reshape(NQ, 4, ROWS, D).transpose(0, 3, 1, 2).reshape(NQ, D, 512), 1.0)
        m = dict(shared)
        m.update({
            "e0t": e0t,
            "ot": q8(On[fr].reshape(FPC * O, D).T, 1.0),
            "ht8": q8(Hn[fr].reshape(FPC * H, D).T, 32.0),
            "h_rm": np.ascontiguousarray(Hn[fr].reshape(FPC * H, D)).astype(NB),
            "scsf": np.ascontiguousarray(np.concatenate(
                [Sc4[fr].T, np.zeros((D, FPC), np.float32), Sf[fr].T], axis=1)).astype(NB),
            "sc4rm": np.ascontiguousarray(Sc4[fr]).astype(np.float32),
            "sfrm": np.ascontiguousarray(Sf[fr]).astype(np.float32),
        })
        in_maps.append(m)
    return in_maps


LAST_RESULT = None


def kernel(**inputs):
    global LAST_RESULT
    if "nc" not in _CACHE:
        _CACHE["nc"] = _build_nc()
    nc = _CACHE["nc"]
    in_maps = _prep_in_maps(inputs)
    trace = os.environ.get("KERNEL_TRACE", "0") == "1"
    res = bass_utils.run_bass_kernel_spmd(
        nc, in_maps, core_ids=list(range(NCORES)), trace=trace)
    LAST_RESULT = res
    out = np.concatenate([res.results[c]["outp"] for c in range(NCORES)], axis=0)
    return np.ascontiguousarray(out.reshape(B, F, D)).astype(np.float32)


# revision 35
# speedup vs baseline: 1.0245x; 1.0214x over previous
"""Trainium2 Bass kernel for nn_Graph_Enhance_model (GNN message passing).

Self-contained: hardcodes shapes B=4,F=32,H=8,O=16,D=2048, 8 cores.
Data-parallel over the 128 (b,f) frames: 16 frames per core.

v2: fp8e4m3 DoubleRow for the edge waves + human GRU (validated: final
rel_err ~2.4e-3 vs 2e-2 budget), algebraic rank-trick for propagation
step 2 (edges after step 1 are w * [msg_e | bcast(msg_n)]; the msg_n
half is rank-16 per frame, so step 2 contracts K=1024 + a rank-16
correction), S-node GRU kept bf16, its input-independent half (whh @
[sc4|sf]) interleaved into Phase A so its weight DMA hides under the
compute-bound wave phase.

Scale conventions (power-of-2, folded into host tensors / activation
scale):
  wcat = 64*[We|Wl1]^T fp8      wave psum (step0) = 64 * true
  e0t  = edges fp8 (x1)         wave psum (step1) = 256 * true
  um1t = 4 * (w1 x msg_e1) fp8  c_sb = 64 * (msg_n @ W_R^T) bf16
  diag64 = 4 * w1 bf16 (mask64 host = 64, times wb0 = w/16)
  ghi/ghh = 64*w^T fp8, msum8 = 32*msum, ht8 = 32*h^T -> psum = 2048x
"""

import os
import sys

for _p in ("/opt/trn_rl_repo", "/opt/pypackages"):
    if _p not in sys.path and os.path.isdir(_p):
        sys.path.append(_p)

import numpy as np
import ml_dtypes

import concourse.bass as bass
import concourse.bacc as bacc
import concourse.tile as tile
import concourse.mybir as mybir
from concourse import bass_utils
from concourse.masks import make_identity

BF16 = mybir.dt.bfloat16
FP8 = mybir.dt.float8e4
F32 = mybir.dt.float32
AF = mybir.ActivationFunctionType
ALU = mybir.AluOpType
AX = mybir.AxisListType
DR = mybir.MatmulPerfMode.DoubleRow

NB = ml_dtypes.bfloat16
N8 = ml_dtypes.float8_e4m3fn
NE3 = ml_dtypes.float8_e3m4
FP8E3 = mybir.dt.float8e3

B, F, H, O, D = 4, 32, 8, 16, 2048
NFRAMES = B * F          # 128
NCORES = 8
FPC = NFRAMES // NCORES  # 16 frames per core
ROWS = H * O             # 128 rows per frame
KC = D // 128            # 16 K-chunks
NQ = FPC // 4            # 4 quads of 4 frames

_CACHE = {}


def q8(x, scale):
    return np.clip(np.asarray(x, np.float32) * scale, -240, 240).astype(N8)


def qe3(x, scale):
    return np.clip(np.asarray(x, np.float32) * scale, -15.5, 15.5).astype(NE3)


def _build_nc():
    nc = bacc.Bacc("TRN2", target_bir_lowering=False, debug=False, num_devices=NCORES)

    def din(name, shape, dt):
        return nc.dram_tensor(name, shape, dt, kind="ExternalInput")

    e0t = din("e0t", [NQ, D, 512], FP8)
    ot = din("ot", [D, FPC * O], FP8)
    wnt = din("wnt", [D, D // 2], FP8)
    wnb64 = din("wnb64", [1, D // 2], BF16)
    wcat = din("wcat", [D, D], FP8)
    bl1td = din("bl1t", [128, 8], F32)
    bet64d = din("bet64", [128, 8], F32)
    bet256d = din("bet256", [128, 8], F32)
    wl2 = din("wl2", [128, 8], BF16)
    mask64d = din("mask64", [64, 512], BF16)
    ht8d = din("ht8", [D, FPC * H], FP8)
    h_rmd = din("h_rm", [FPC * H, D], BF16)
    pmatd = din("pmat", [FPC * H, FPC], BF16)
    ghi = din("ghi", [D, 3 * D], FP8)
    ghib = din("ghib", [1, 3 * D], BF16)
    ghh = din("ghh", [D, 3 * D], FP8)
    ghhb = din("ghhb", [1, 3 * D], BF16)
    scsf = din("scsf", [D, 3 * FPC], BF16)
    sc4rmd = din("sc4rm", [FPC, D], F32)
    sfrmd = din("sfrm", [FPC, D], F32)
    gsi = din("gsi", [D, 3 * D], FP8E3)
    gsib = din("gsib", [1, 3 * D], BF16)
    gsh = din("gsh", [D, 3 * D], BF16)
    gshb = din("gshb", [1, 3 * D], BF16)
    outp = nc.dram_tensor("outp", [FPC, D], F32, kind="ExternalOutput")

    from contextlib import ExitStack

    with tile.TileContext(nc) as tc, ExitStack() as ctx:
        glob = ctx.enter_context(tc.tile_pool(name="glob", bufs=1))

        ones_b = glob.tile([1, 512], BF16)
        nc.vector.memset(ones_b, 1.0)
        ident16 = glob.tile([16, 16], BF16)
        make_identity(nc, ident16)

        wl2_sb = glob.tile([128, 8], BF16)
        bl1t_sb = glob.tile([128, 8], F32)
        bet64_sb = glob.tile([128, 8], F32)
        bet256_sb = glob.tile([128, 8], F32)
        mask64_sb = glob.tile([64, 512], BF16)

        msum8 = glob.tile([128, KC, FPC * H], FP8)      # 32 * msum
        ht8_sb = glob.tile([128, KC, FPC * H], FP8)     # 32 * h^T
        scsf_b = glob.tile([128, KC, 3 * FPC], BF16)    # [S_C4^T | 0 | S_f^T]
        gh1_sb = glob.tile([16, 12, 512], BF16)         # whh@sc4 + bhh
        gh2_sb = glob.tile([16, 12, 512], BF16)         # whh@sf + bhh
        ah_sb = glob.tile([128, KC, FPC], FP8E3)        # 2 * All_human^T

        with (
            tc.tile_pool(name="pwcat", bufs=1) as pwcat,
            tc.tile_pool(name="pa", bufs=2) as pa,
            tc.tile_pool(name="pa1", bufs=1) as pa1,
            tc.tile_pool(name="pc0w", bufs=2) as pc0w,
        ):
            wcat_sb = pwcat.tile([128, KC, D], FP8)
            msgn_sb = pa1.tile([128, 8, FPC * O], BF16)   # true msg_n^T (+bn)
            c_sb = pa1.tile([64, NQ, D], BF16)            # 64 * msg_n @ W_R^T
            msum_f = pa1.tile([128, KC, FPC * H], F32)    # true M_sum2^T

            # ---------------- Phase 0: msg_n^T = Wn @ O^T + bn (fp8 DR) ----------------
            with (
                tc.tile_pool(name="p0", bufs=1) as p0,
                tc.tile_pool(name="p0ps", bufs=4, space="PSUM") as p0ps,
            ):
                # DMA issue order = critical-path order: phase-0 inputs, then
                # first edge quad, then wcat (chunked so waves start early),
                # then everything else.
                ot_sb = p0.tile([128, KC, FPC * O], FP8)
                nc.sync.dma_start(out=ot_sb, in_=ot.ap().rearrange("(kc p) n -> p kc n", p=128))
                wn_sb = p0.tile([128, KC, D // 2], FP8)
                nc.sync.dma_start(out=wn_sb, in_=wnt.ap().rearrange("(kc p) m -> p kc m", p=128))
                wnb_sb = p0.tile([1, D // 2], BF16)
                nc.sync.dma_start(out=wnb_sb, in_=wnb64.ap())
                msgn8 = p0.tile([128, 8, FPC * O], FP8)   # 16 * msg_n
                xqs = {}
                for q in range(2):
                    xqs[q] = pa.tile([128, KC, 512], FP8, tag="xq", name="xq")
                    nc.sync.dma_start(out=xqs[q],
                                      in_=e0t.ap()[q].rearrange("(kc p) n -> p kc n", p=128))
                for k2 in range(8):
                    nc.sync.dma_start(
                        out=wcat_sb[:, 2 * k2:2 * k2 + 2, :],
                        in_=wcat.ap()[256 * k2:256 * (k2 + 1), :]
                        .rearrange("(kc p) m -> p kc m", p=128))
                nc.sync.dma_start(out=wl2_sb, in_=wl2.ap())
                nc.sync.dma_start(out=bl1t_sb, in_=bl1td.ap())
                nc.sync.dma_start(out=bet64_sb, in_=bet64d.ap())
                nc.sync.dma_start(out=bet256_sb, in_=bet256d.ap())
                nc.sync.dma_start(out=mask64_sb, in_=mask64d.ap())
                nc.sync.dma_start(out=ht8_sb, in_=ht8d.ap().rearrange("(kc p) n -> p kc n", p=128))
                nc.sync.dma_start(out=scsf_b, in_=scsf.ap().rearrange("(kc p) n -> p kc n", p=128))
                for mt in range(8):
                    pm = p0ps.tile([128, FPC * O], F32, tag="pm")
                    for k2 in range(8):
                        nc.tensor.matmul(pm, lhsT=wn_sb[:, 2 * k2:2 * k2 + 2, mt * 128:(mt + 1) * 128],
                                         rhs=ot_sb[:, 2 * k2:2 * k2 + 2, :],
                                         start=(k2 == 0), stop=False, perf_mode=DR)
                    nc.tensor.matmul(pm, lhsT=wnb_sb[0:1, mt * 128:(mt + 1) * 128],
                                     rhs=ones_b[0:1, 0:FPC * O], start=False, stop=True)
                    nc.scalar.activation(msgn_sb[:, mt, :], pm, AF.Copy, scale=1.0 / 64)
                    nc.scalar.activation(msgn8[:, mt, :], pm, AF.Copy, scale=1.0 / 4)

                # c = msg_n @ [We_R | Wl1_R]^T  (per-q layout, 64*c in bf16)
                for q in range(NQ):
                    for mtile in range(4):
                        pcp = p0ps.tile([64, 512], F32, tag="pc")
                        for k2 in range(4):
                            nc.tensor.matmul(
                                pcp,
                                lhsT=msgn8[:, 2 * k2:2 * k2 + 2, q * 64:(q + 1) * 64],
                                rhs=wcat_sb[:, 8 + 2 * k2:8 + 2 * k2 + 2,
                                            mtile * 512:(mtile + 1) * 512],
                                start=(k2 == 0), stop=(k2 == 3), perf_mode=DR)
                        nc.scalar.activation(c_sb[:, q, mtile * 512:(mtile + 1) * 512],
                                             pcp, AF.Copy, scale=1.0 / 16)

            # ---------------- Phase A: 2 propagation steps + interleaved C0 ----------------
            # C0 = S-GRU whh @ [sc4 | sf] (input-independent), spread across
            # slots; back-loaded so its PE work covers the q=3 DVE tail.
            c0_sched = [[], [0], [1], [2], [3, 4], [5, 6], [7, 8], [9, 10, 11]]

            def emit_c0(j):
                wsh = pc0w.tile([128, KC, 512], BF16, tag="wsh")
                nc.sync.dma_start(out=wsh, in_=gsh.ap()[:, j * 512:(j + 1) * 512]
                                  .rearrange("(kc p) m -> p kc m", p=128))
                bsh = pc0w.tile([1, 512], BF16, tag="bsh")
                nc.sync.dma_start(out=bsh, in_=gshb.ap()[:, j * 512:(j + 1) * 512])
                PH = pc0ps.tile([48, 512], F32, tag="PH")
                for kc in range(KC):
                    nc.tensor.matmul(PH, lhsT=scsf_b[:, kc, :], rhs=wsh[:, kc, :],
                                     start=(kc == 0), stop=False)
                nc.tensor.matmul(PH, lhsT=ones_b[0:1, 0:48], rhs=bsh[0:1, :],
                                 start=False, stop=True)
                nc.scalar.copy(gh1_sb[:, j, :], PH[0:16, :])
                nc.scalar.copy(gh2_sb[:, j, :], PH[32:48, :])

            with tc.tile_pool(name="paps", bufs=4, space="PSUM") as paps, \
                 tc.tile_pool(name="papss", bufs=1, space="PSUM") as papss, \
                 tc.tile_pool(name="pc0ps", bufs=1, space="PSUM") as pc0ps:
                for q in range(NQ):
                    if q in xqs:
                        xq = xqs[q]
                    else:
                        xq = pa.tile([128, KC, 512], FP8, tag="xq")
                        nc.sync.dma_start(out=xq, in_=e0t.ap()[q].rearrange("(kc p) n -> p kc n", p=128))
                    um1t = pa1.tile([128, 8, 512], FP8, tag="um1t")
                    diag64 = pa1.tile([64, 512], BF16, tag="diag")
                    for step in range(2):
                        psc = 1.0 / 64 if step == 0 else 1.0 / 256

                        def wave_mms(pt, mcol):
                            """full K-contraction for output cols mcol*128
                            into psum pt (start..stop)."""
                            if step == 0:
                                for k2 in range(8):
                                    nc.tensor.matmul(
                                        pt, lhsT=wcat_sb[:, 2 * k2:2 * k2 + 2,
                                                         mcol * 128:(mcol + 1) * 128],
                                        rhs=xq[:, 2 * k2:2 * k2 + 2, :],
                                        start=(k2 == 0), stop=(k2 == 7), perf_mode=DR)
                            else:
                                for k2 in range(4):
                                    nc.tensor.matmul(
                                        pt, lhsT=wcat_sb[:, 2 * k2:2 * k2 + 2,
                                                         mcol * 128:(mcol + 1) * 128],
                                        rhs=um1t[:, 2 * k2:2 * k2 + 2, :],
                                        start=(k2 == 0), stop=False, perf_mode=DR)
                                nc.tensor.matmul(
                                    pt, lhsT=c_sb[:, q, mcol * 128:(mcol + 1) * 128],
                                    rhs=diag64, start=False, stop=True)

                        # --- a-wave: relu(X @ Wl1^T + bl1), transposed ---
                        relu_sb = pa1.tile([128, 8, 512], BF16, tag="relu")
                        for mt in range(8, 16):
                            pw_a = paps.tile([128, 512], F32, tag="wave")
                            wave_mms(pw_a, mt)
                            nc.scalar.activation(relu_sb[:, mt - 8, :], pw_a, AF.Relu,
                                                 bias=bl1t_sb[:, mt - 8:mt - 7], scale=psc)
                        # --- logits + softmax over o (groups of 16) ---
                        pl = papss.tile([1, 512], F32, tag="pl")
                        for kc2 in range(8):
                            nc.tensor.matmul(pl, lhsT=wl2_sb[:, kc2:kc2 + 1],
                                             rhs=relu_sb[:, kc2, :], start=(kc2 == 0), stop=(kc2 == 7))
                        pl3 = pl.rearrange("o (g i) -> o g i", i=16)
                        mx = pa1.tile([1, 32], F32, tag="mx")
                        nc.vector.reduce_max(mx, pl3, axis=AX.X)
                        sub = pa1.tile([1, 512], F32, tag="sub")
                        nc.vector.tensor_tensor(sub.rearrange("o (g i) -> o g i", i=16), pl3,
                                                mx.broadcast_to((1, 32, 16)), op=ALU.subtract)
                        nc.scalar.activation(sub, sub, AF.Exp)
                        ex3 = sub.rearrange("o (g i) -> o g i", i=16)
                        sm = pa1.tile([1, 32], F32, tag="sm")
                        nc.vector.reduce_sum(sm, ex3, axis=AX.X)
                        rs = pa1.tile([1, 32], F32, tag="rs")
                        nc.vector.reciprocal(rs, sm)
                        w_sb = pa1.tile([1, 512], BF16, tag="w")
                        nc.vector.tensor_tensor(w_sb.rearrange("o (g i) -> o g i", i=16), ex3,
                                                rs.broadcast_to((1, 32, 16)), op=ALU.mult)
                        # --- msg_e wave; w-broadcast emitted after 2 groups ---
                        e_ps = []
                        wb0 = pa1.tile([128, 512], F32, tag="wb0")
                        wb1 = pa1.tile([128, 512], F32, tag="wb1")

                        def combine(cmt, pe):
                            if step == 0:
                                nc.vector.scalar_tensor_tensor(
                                    out=um1t[:, cmt, :], in0=pe,
                                    scalar=bet64_sb[:, cmt:cmt + 1], in1=wb0,
                                    op0=ALU.add, op1=ALU.mult)
                            else:
                                tmp = pa1.tile([128, 512], F32, tag="tmp")
                                nc.vector.scalar_tensor_tensor(
                                    out=tmp, in0=pe,
                                    scalar=bet256_sb[:, cmt:cmt + 1], in1=wb1,
                                    op0=ALU.add, op1=ALU.mult)
                                nc.vector.reduce_sum(
                                    msum_f[:, cmt, q * 32:(q + 1) * 32],
                                    tmp.rearrange("p (f h o) -> p f h o", f=4, h=8),
                                    axis=AX.X)

                        for mt in range(8):
                            pe = paps.tile([128, 512], F32, tag="wave")
                            wave_mms(pe, mt)
                            e_ps.append(pe)
                            if mt == 1:
                                pw_b = papss.tile([128, 512], F32, tag="pw")
                                nc.tensor.matmul(pw_b, lhsT=ones_b[0:1, 0:128], rhs=w_sb,
                                                 start=True, stop=True)
                                if step == 0:
                                    nc.scalar.activation(wb0, pw_b, AF.Copy, scale=1.0 / 16)
                                    nc.vector.tensor_tensor(diag64, mask64_sb, wb0[0:64, :],
                                                            op=ALU.mult)
                                else:
                                    nc.scalar.activation(wb1, pw_b, AF.Copy, scale=1.0 / 4096)
                                    nc.scalar.activation(wb0, pw_b, AF.Copy, scale=1.0 / 16)
                            if mt >= 1:
                                for cmt in ([0, 1] if mt == 1 else [mt]):
                                    combine(cmt, e_ps[cmt])
                        if step == 1:
                            # msg_n half of M_sum: sum_o (w2/16) * msg_n
                            wb4 = wb0.rearrange("p (f h o) -> p f h o", f=4, h=8)
                            for j in range(8):
                                mt = 8 + j
                                base = msgn_sb[:, j, q * 64:(q + 1) * 64]
                                mn_bc = bass.AP(tensor=base.tensor, offset=base.offset,
                                                ap=[list(base.ap[0]), [16, 4], [0, 8], [1, 16]])
                                tmp2 = pa1.tile([128, 512], F32, tag="tmp2")
                                nc.vector.tensor_tensor(
                                    tmp2.rearrange("p (f h o) -> p f h o", f=4, h=8),
                                    mn_bc, wb4, op=ALU.mult)
                                nc.vector.reduce_sum(
                                    msum_f[:, mt, q * 32:(q + 1) * 32],
                                    tmp2.rearrange("p (f h o) -> p f h o", f=4, h=8),
                                    axis=AX.X)
                        # interleave C0 blocks (keeps gsh streaming during A)
                        for j in c0_sched[q * 2 + step]:
                            emit_c0(j)
                nc.vector.tensor_scalar_mul(msum8, msum_f, 32.0)

        # ---------------- Phases B+C scope ----------------
        pbc = ctx.enter_context(tc.tile_pool(name="pbc", bufs=1))
        # S-GRU wih, e3m4, cached whole in SBUF: used by both C1 and C2
        gsi_all = pbc.tile([128, KC, 3 * D], FP8E3)

        def prefetch_gsi(j):
            nc.sync.dma_start(out=gsi_all[:, :, j * 512:(j + 1) * 512],
                              in_=gsi.ap()[:, j * 512:(j + 1) * 512]
                              .rearrange("(kc p) m -> p kc m", p=128))

        # ---------------- Phase B: human GRU (fp8 DoubleRow, row-major) ----------------
        with (
            tc.tile_pool(name="pbw", bufs=3) as pbw,
            tc.tile_pool(name="pbb", bufs=3) as pbb,
            tc.tile_pool(name="pb1", bufs=1) as pb1,
            tc.tile_pool(name="pbps", bufs=1, space="PSUM") as pbps,
            tc.tile_pool(name="pbps2", bufs=2, space="PSUM") as pbps2,
        ):
            NR = FPC * H  # 128 rows
            h_rm = pb1.tile([NR, D], BF16)
            nc.sync.dma_start(out=h_rm, in_=h_rmd.ap())
            pmat_sb = pb1.tile([NR, FPC], BF16)
            nc.sync.dma_start(out=pmat_sb, in_=pmatd.ap())
            hum_b = pb1.tile([NR, D], BF16)

            def gh_block(j, pt, use_i, use_h):
                """accumulate 2048*(gi and/or gh) for gate block j into psum
                pt, row-major [128 rows, 512 gates], fp8 DoubleRow."""
                ops = []
                if use_h:
                    wb_t = pbw.tile([128, KC, 512], FP8, tag="bwh")
                    nc.sync.dma_start(out=wb_t, in_=ghh.ap()[:, j * 512:(j + 1) * 512]
                                      .rearrange("(kc p) m -> p kc m", p=128))
                    bb = pbb.tile([1, 512], BF16, tag="bbh")
                    nc.sync.dma_start(out=bb, in_=ghhb.ap()[:, j * 512:(j + 1) * 512])
                    ops += [(wb_t, ht8_sb, k2) for k2 in range(8)] + [(bb, None, None)]
                if use_i:
                    wi_t = pbw.tile([128, KC, 512], FP8, tag="bwi")
                    nc.sync.dma_start(out=wi_t, in_=ghi.ap()[:, j * 512:(j + 1) * 512]
                                      .rearrange("(kc p) m -> p kc m", p=128))
                    bi = pbb.tile([1, 512], BF16, tag="bbi")
                    nc.sync.dma_start(out=bi, in_=ghib.ap()[:, j * 512:(j + 1) * 512])
                    ops += [(wi_t, msum8, k2) for k2 in range(8)] + [(bi, None, None)]
                for idx, (w, x, k2) in enumerate(ops):
                    st, sp = idx == 0, idx == len(ops) - 1
                    if x is None:
                        nc.tensor.matmul(pt, lhsT=ones_b[0:1, 0:128], rhs=w[0:1, :],
                                         start=st, stop=sp)
                    else:
                        nc.tensor.matmul(pt, lhsT=x[:, 2 * k2:2 * k2 + 2, :],
                                         rhs=w[:, 2 * k2:2 * k2 + 2, :],
                                         start=st, stop=sp, perf_mode=DR)

            for t in range(4):
                cols = slice(t * 512, (t + 1) * 512)
                # h-only group first: it has no msum dependency, so the PE can
                # chew on it while the q=3 M_sum combines drain on DVE
                p_hn = pbps.tile([NR, 512], F32, tag="phn")
                gh_block(8 + t, p_hn, False, True)
                p_r = pbps.tile([NR, 512], F32, tag="pr")
                gh_block(t, p_r, True, True)
                p_z = pbps.tile([NR, 512], F32, tag="pz")
                gh_block(4 + t, p_z, True, True)
                p_in = pbps.tile([NR, 512], F32, tag="pin")
                gh_block(8 + t, p_in, True, False)
                for j in (3 * t, 3 * t + 1, 3 * t + 2):
                    prefetch_gsi(j)
                r_sb = pb1.tile([NR, 512], F32, tag="r")
                nc.scalar.activation(r_sb, p_r, AF.Sigmoid, scale=1.0 / 2048)
                z_sb = pb1.tile([NR, 512], F32, tag="z")
                nc.scalar.activation(z_sb, p_z, AF.Sigmoid, scale=1.0 / 2048)
                t1 = pb1.tile([NR, 512], F32, tag="t1")
                nc.vector.tensor_tensor(t1, r_sb, p_hn, op=ALU.mult)
                t2 = pb1.tile([NR, 512], F32, tag="r", name="t2")
                nc.vector.tensor_tensor(t2, t1, p_in, op=ALU.add)
                n_sb = pb1.tile([NR, 512], F32, tag="n")
                nc.scalar.activation(n_sb, t2, AF.Tanh, scale=1.0 / 2048)
                t3 = pb1.tile([NR, 512], F32, tag="t3")
                nc.vector.tensor_tensor(t3, h_rm[:, cols], n_sb, op=ALU.subtract)
                t4 = pb1.tile([NR, 512], F32, tag="t1", name="t4")
                nc.vector.tensor_tensor(t4, z_sb, t3, op=ALU.mult)
                nc.vector.tensor_tensor(hum_b[:, cols], n_sb, t4, op=ALU.add)
            # All_human^T chunks via PE: ah[c] = hum[:, c-chunk].T @ pmat
            for c in range(KC):
                pah = pbps2.tile([128, FPC], F32, tag="pah")
                nc.tensor.matmul(pah, lhsT=hum_b[:, c * 128:(c + 1) * 128], rhs=pmat_sb,
                                 start=True, stop=True)
                nc.scalar.activation(ah_sb[:, c, :], pah, AF.Copy, scale=2.0)

        # ---------------- Phase C: two S-node GRUs (wih e3m4 cached) ----------------
        with (
            tc.tile_pool(name="pc1", bufs=1) as pc1,
            tc.tile_pool(name="pcsm", bufs=1) as pcsm,
            tc.tile_pool(name="pcps", bufs=2, space="PSUM") as pcps,
            tc.tile_pool(name="pctps", bufs=2, space="PSUM") as pctps,
        ):
            gsib_sb = pc1.tile([1, 3 * D], BF16)
            nc.sync.dma_start(out=gsib_sb, in_=gsib.ap())
            sc4rm_sb = pc1.tile([FPC, D], F32)
            nc.sync.dma_start(out=sc4rm_sb, in_=sc4rmd.ap())
            sfrm_sb = pc1.tile([FPC, D], F32)
            nc.sync.dma_start(out=sfrm_sb, in_=sfrmd.ap())
            g1_sb = pc1.tile([16, 12, 512], BF16)
            g2_sb = pc1.tile([16, 12, 512], BF16)
            s1_sb = pc1.tile([16, D], BF16)
            s1t = pc1.tile([128, KC, 16], FP8E3)
            out32 = pc1.tile([FPC, D], F32)

            def sgru_wave(j, lhs_kc):
                PZ = pcps.tile([16, 512], F32, tag="PC")
                for kc in range(KC):
                    nc.tensor.matmul(PZ, lhsT=lhs_kc[:, kc, :],
                                     rhs=gsi_all[:, kc, j * 512:(j + 1) * 512],
                                     start=(kc == 0), stop=False)
                nc.tensor.matmul(PZ, lhsT=ones_b[0:1, 0:16],
                                 rhs=gsib_sb[0:1, j * 512:(j + 1) * 512],
                                 start=False, stop=True)
                return PZ

            # j order groups (r_t, z_t, n_t) so col-block t's elementwise can
            # start after its triple; transposes/output interleave under PE
            jorder = [0, 4, 8, 1, 5, 9, 2, 6, 10, 3, 7, 11]

            def s_combine(step, j, PZ):
                gx_sb = g1_sb if step == 0 else g2_sb
                gh_sb = gh1_sb if step == 0 else gh2_sb
                if j < 8:
                    nc.vector.scalar_tensor_tensor(out=gx_sb[:, j, :], in0=PZ,
                                                   scalar=1.0 / 128, in1=gh_sb[:, j, :],
                                                   op0=ALU.mult, op1=ALU.add)
                else:
                    nc.scalar.activation(gx_sb[:, j, :], PZ, AF.Copy, scale=1.0 / 128)

            def s_elem(step, t):
                cols = slice(t * 512, (t + 1) * 512)
                gx_sb = g1_sb if step == 0 else g2_sb
                gh_sb = gh1_sb if step == 0 else gh2_sb
                hsb = sc4rm_sb if step == 0 else sfrm_sb
                dst = s1_sb if step == 0 else out32
                z1 = pcsm.tile([16, 512], F32, tag="z1", name="z1")
                nc.scalar.activation(z1, gx_sb[:, 4 + t, :], AF.Sigmoid)
                r1 = pcsm.tile([16, 512], F32, tag="r1", name="r1")
                nc.scalar.activation(r1, gx_sb[:, t, :], AF.Sigmoid)
                u1 = pcsm.tile([16, 512], F32, tag="u1", name="u1")
                nc.vector.tensor_tensor(u1, r1, gh_sb[:, 8 + t, :], op=ALU.mult)
                u2 = pcsm.tile([16, 512], F32, tag="u2", name="u2")
                nc.vector.tensor_tensor(u2, u1, gx_sb[:, 8 + t, :], op=ALU.add)
                n1 = pcsm.tile([16, 512], F32, tag="n1", name="n1")
                nc.scalar.activation(n1, u2, AF.Tanh)
                u3 = pcsm.tile([16, 512], F32, tag="u3", name="u3")
                nc.vector.tensor_tensor(u3, hsb[:, cols], n1, op=ALU.subtract)
                u4 = pcsm.tile([16, 512], F32, tag="u4", name="u4")
                nc.vector.tensor_tensor(u4, z1, u3, op=ALU.mult)
                nc.vector.tensor_tensor(dst[:, cols], n1, u4, op=ALU.add)
                if step == 0:
                    # transpose finished s1 cols -> s1t (e3m4, 2*s1)
                    for c in range(4 * t, 4 * t + 4):
                        ptp = pctps.tile([128, 16], BF16, tag="tp", name="tp")
                        nc.tensor.transpose(ptp, s1_sb[:, c * 128:(c + 1) * 128], ident16)
                        nc.scalar.activation(s1t[:, c, :], ptp, AF.Copy, scale=2.0)
                else:
                    nc.sync.dma_start(out=outp.ap()[:, cols], in_=out32[:, cols])

            # step 1: gi1 = wih @ All_human + bih (psum = 128 * gi1), then s1
            for idx, j in enumerate(jorder):
                PZ = sgru_wave(j, ah_sb)
                s_combine(0, j, PZ)
                if idx % 3 == 2:
                    s_elem(0, idx // 3)
            # step 2: gi2 = wih @ s1 + bih; combine with gh2 -> out
            for idx, j in enumerate(jorder):
                PZ = sgru_wave(j, s1t)
                s_combine(1, j, PZ)
                if idx % 3 == 2:
                    s_elem(1, idx // 3)

    nc.compile()
    return nc


def _prep_in_maps(inputs):
    E = np.ascontiguousarray(inputs["H_O_edges"].reshape(NFRAMES, ROWS, D))
    On = inputs["O_nodes"].reshape(NFRAMES, O, D)
    Hn = inputs["H_nodes"].reshape(NFRAMES, H, D)
    Sc4 = inputs["S_node_C4"].reshape(NFRAMES, D)
    Sf = np.ascontiguousarray(inputs["final_S_node"].transpose(0, 2, 1)).reshape(NFRAMES, D)

    mask64 = np.zeros((64, 512), np.float32)
    for f in range(4):
        for o in range(O):
            for h in range(H):
                mask64[f * 16 + o, f * 128 + h * 16 + o] = 64.0

    shared = {
        "wcat": q8(np.concatenate([inputs["We"], inputs["Wl1"]], axis=0).T, 64.0),
        "bl1t": np.ascontiguousarray(inputs["bl1"].reshape(8, 128).T).astype(np.float32),
        "bet64": np.ascontiguousarray(inputs["be"].reshape(8, 128).T).astype(np.float32) * 64.0,
        "bet256": np.ascontiguousarray(inputs["be"].reshape(8, 128).T).astype(np.float32) * 256.0,
        "mask64": mask64.astype(NB),
        "pmat": np.ascontiguousarray(np.kron(np.eye(FPC), np.ones((H, 1))) / H).astype(NB),
        "wnt": q8(inputs["Wn"].T, 64.0),
        "wnb64": (inputs["bn"][None, :] * 64.0).astype(NB),
        "wl2": np.ascontiguousarray(inputs["Wl2"][0].reshape(8, 128).T).astype(NB),
        "ghi": q8(inputs["gh_wih"].T, 64.0),
        "ghib": (inputs["gh_bih"][None, :] * 2048.0).astype(NB),
        "ghh": q8(inputs["gh_whh"].T, 64.0),
        "ghhb": (inputs["gh_bhh"][None, :] * 2048.0).astype(NB),
        "gsi": qe3(np.ascontiguousarray(inputs["gs_wih"].T), 64.0),
        "gsib": (inputs["gs_bih"][None, :] * 128.0).astype(NB),
        "gsh": np.ascontiguousarray(inputs["gs_whh"].T).astype(NB),
        "gshb": inputs["gs_bhh"][None, :].astype(NB),
    }

    in_maps = []
    for c in range(NCORES):
        fr = slice(c * FPC, (c + 1) * FPC)
        Ec = E[fr]  # [16, 128, 2048]
        e0t = q8(Ec.reshape(NQ, 4, ROWS, D).transpose(0, 3, 1, 2).reshape(NQ, D, 512), 1.0)
        m = dict(shared)
        m.update({
            "e0t": e0t,
            "ot": q8(On[fr].reshape(FPC * O, D).T, 1.0),
            "ht8": q8(Hn[fr].reshape(FPC * H, D).T, 32.0),
            "h_rm": np.ascontiguousarray(Hn[fr].reshape(FPC * H, D)).astype(NB),
            "scsf": np.ascontiguousarray(np.concatenate(
                [Sc4[fr].T, np.zeros((D, FPC), np.float32), Sf[fr].T], axis=1)).astype(NB),
            "sc4rm": np.ascontiguousarray(Sc4[fr]).astype(np.float32),
            "sfrm": np.ascontiguousarray(Sf[fr]).astype(np.float32),
        })
        in_maps.append(m)
    return in_maps


LAST_RESULT = None


def kernel(**inputs):
    global LAST_RESULT
    if "nc" not in _CACHE:
        _CACHE["nc"] = _build_nc()
    nc = _CACHE["nc"]
    in_maps = _prep_in_maps(inputs)
    trace = os.environ.get("KERNEL_TRACE", "0") == "1"
    res = bass_utils.run_bass_kernel_spmd(
        nc, in_maps, core_ids=list(range(NCORES)), trace=trace)
    LAST_RESULT = res
    out = np.concatenate([res.results[c]["outp"] for c in range(NCORES)], axis=0)
    return np.ascontiguousarray(out.reshape(B, F, D)).astype(np.float32)
